# revision 10
# baseline (speedup 1.0000x reference)
"""Trainium2 Bass kernel for nn_ATAC_Encoder (GCN message passing), 8 cores.

Math (reference):
    f  = LayerNorm(x @ W1.T + b1) * ln1_g + ln1_b
    sc = BatchNorm(x @ sc_W.T + sc_b) * bn_g + bn_b      (batch stats over nodes)
    h  = leaky_relu(f + sc, 0.01)
    g  = GCNConv(h, edge_index, gcn_W, gcn_b)            (sym-norm, self-loops)
    out = leaky_relu(LayerNorm(g) * ln2_g + ln2_b, 0.01)

Distribution: nodes block-sharded across 8 NeuronCores (6250 each, padded to
6272 = 49*128). Pipeline per core:
  pass1a: ysc = x @ scW.T, BatchNorm partial sums accumulated on the vector
          engine (S_acc/Q_acc) then reduced with two f32r ones-matmuls; the
          2KB AllReduce is kicked immediately so it overlaps pass1b.
  pass1b: y1 = x @ W1.T (x re-streamed from HBM), LayerNorm per block.
  pass2:  h = leaky_relu(ysc*A + f + Ball), hT via PE transpose, xw = hT @
          gcnW.T, scale by dinv, store; the scaled table is AllGathered in 4
          node-quarter chunks (small quarter first so the collective chain
          starts early).
  pass3:  per-edge source rows are gathered from the table with SWDGE
          dma_gather calls packed into fixed-size index streams (one lo / one
          hi stream per core, int16 table-half split), with gather counts in
          preloaded constant registers (no per-call register loads);
          aggregation via selector-matrix matmuls on the tensor engine:
          acc[t,:] += S_g.T @ msgs_g, then dinv scale, +gcn_b, LayerNorm,
          leaky_relu.
"""

import os
import numpy as np
from contextlib import ExitStack

import concourse.bass as bass
import concourse.mybir as mybir
import concourse.tile as tile
from concourse import bacc
from concourse.bass_utils import run_bass_kernel_spmd

F32 = mybir.dt.float32
F32R = mybir.dt.float32r
BF16 = mybir.dt.bfloat16
I16 = mybir.dt.int16

LN_EPS = 1e-5
BN_EPS = 1e-5
SLOPE = 0.01
GATHER_SUB = int(os.environ.get("GNN_SUB", "896"))  # idxs per gather call
MSG_RING = int(os.environ.get("GNN_RING", "20"))    # gather ring slots
USE_LRELU = os.environ.get("GNN_NO_LRELU", "") != "1"  # pass-2 only (act table)
USE_RSQRT = os.environ.get("GNN_RSQRT", "") == "1"  # blocked: accuracy issues
ISEQ_SWAP = os.environ.get("GNN_ISEQ_SWAP", "") == "1"
QS_PATTERN = os.environ.get("GNN_QS", "4,13,15,17")


def _cols_wrap(v: np.ndarray, L: int) -> np.ndarray:
    """[cnt] col list -> [128, L/128] fp32 (slot m -> [m%128, m//128]), pad -1."""
    a = np.full(L, -1.0, np.float32)
    a[: len(v)] = v
    return a.reshape(L // 128, 128).T


def preprocess(x, edge_index, W1, b1, ln1_g, ln1_b, sc_W, sc_b, bn_g, bn_b,
               gcn_W, gcn_b, ln2_g, ln2_b, n_cores=8, split=32768):
    """Shard inputs; returns (in_maps, L_lo, L_hi, cfg)."""
    x = np.asarray(x, np.float32)
    ei = np.asarray(edge_index)
    N, D0 = x.shape
    D1 = W1.shape[0]
    D2 = gcn_W.shape[0]
    P = n_cores
    NC = N // P
    NPAD = ((NC + 127) // 128) * 128
    NB = NPAD // 128
    TBL = P * NPAD

    # node-quarter chunking of the AllGather; table rows quarter-major then
    # core-major. Small first quarter so the AG chain starts early.
    szs = [int(s) for s in QS_PATTERN.split(",")]
    assert sum(szs) == NB, (szs, NB)
    nq = len(szs)
    qs = [0]
    for z in szs:
        qs.append(qs[-1] + z)
    qsz = np.array(szs)
    qbase = np.cumsum([0] + [P * int(s) * 128 for s in qsz])
    q_of_block = np.repeat(np.arange(nq), qsz)
    qs_arr = np.array(qs[:nq])

    # self-loops are handled locally in pass 3 (identity matmul of the
    # block's own xws rows); only real edges go through the gather. The
    # degree still counts the self-loop (A+I normalization).
    loops = np.arange(N, dtype=np.int64)
    row = ei[0].astype(np.int64)
    col = ei[1].astype(np.int64)

    deg = np.bincount(np.concatenate([col, loops]), minlength=N).astype(np.float64)
    dinv = (1.0 / np.sqrt(deg)).astype(np.float32)  # deg >= 1 (self-loops)

    sown = row // NC
    sloc = row - sown * NC
    sblk = sloc >> 7
    sq = q_of_block[sblk]
    trow = qbase[sq] + sown * (qsz[sq] * 128) + (sloc - qs_arr[sq] * 128)
    owner = col // NC
    lcol = col - owner * NC

    # bucket edges per (core, target block, lo/hi table half)
    per = []
    for c in range(P):
        m = owner == c
        tr, lc = trow[m], lcol[m]
        blk = lc >> 7
        hi = (tr >= split).astype(np.int64)
        order = np.lexsort((tr, hi, blk))
        tr, lc, blk, hi = tr[order], lc[order], blk[order], hi[order]
        bounds = np.searchsorted(blk * 2 + hi, np.arange(2 * NB + 1))
        blocks = []
        for b in range(NB):
            lo_s, lo_e = bounds[2 * b], bounds[2 * b + 1]
            hi_s, hi_e = bounds[2 * b + 1], bounds[2 * b + 2]
            blocks.append((tr[lo_s:lo_e], tr[hi_s:hi_e] - split,
                           lc[lo_s:lo_e] - b * 128, lc[hi_s:hi_e] - b * 128))
        per.append(blocks)

    def rup128(n):
        return ((n + 127) // 128) * 128

    L_lo = [rup128(max(len(per[c][b][0]) for c in range(P))) for b in range(NB)]
    L_hi = [rup128(max(len(per[c][b][1]) for c in range(P))) for b in range(NB)]
    TL_lo = sum(L_lo)
    TL_hi = sum(L_hi)

    TI = (TL_lo + TL_hi) // 16
    TG = (TL_lo + TL_hi) // 128

    from ml_dtypes import bfloat16

    KC0 = D0 // 128
    w1t = np.ascontiguousarray(W1.T).astype(bfloat16)      # [D0, D1]
    scwt = np.ascontiguousarray(sc_W.T).astype(bfloat16)   # [D0, D1]
    gcnwt = np.ascontiguousarray(gcn_W.T).astype(bfloat16)  # [D1, D2]
    Gmax = max((L_lo[b] + L_hi[b]) // 128 for b in range(NB))
    iota = np.broadcast_to(np.arange(128, dtype=np.float32), (128, 128))
    iota_big = np.ascontiguousarray(np.tile(iota, (1, Gmax))).astype(bfloat16)
    ident = np.eye(128, dtype=np.float32).astype(bfloat16)
    ones = np.ones((128, 1), np.float32)

    vec = lambda v: np.asarray(v, np.float32).reshape(1, -1)

    in_maps = []
    for c in range(P):
        xp = np.zeros((D0, NPAD), np.float32)
        xp[:, :NC] = x[c * NC:(c + 1) * NC].T
        # pretile: [NB, p(d0 in k-chunk), k, n] so each chunk is one
        # contiguous [128, KC*128] DMA
        xp = np.ascontiguousarray(
            xp.reshape(KC0, 128, NB, 128).transpose(2, 1, 0, 3)).astype(bfloat16)

        dv = np.zeros(NPAD, np.float32)
        dv[:NC] = dinv[c * NC:(c + 1) * NC]
        dvT = np.ascontiguousarray(dv.reshape(NB, 128).T)  # [128, NB]

        # flat per-half index streams (block-major, each block 128-padded,
        # pad slots gather row 0) + cols (pad -1 -> S column of zeros)
        idx_lo = np.zeros(TL_lo, np.int16)
        idx_hi = np.zeros(TL_hi, np.int16)
        cols_all = np.full((128, TG), -1.0, np.float32)
        olo = ohi = og = 0
        for b in range(NB):
            lo_t, hi_t, lo_c, hi_c = per[c][b]
            ll, lh = L_lo[b], L_hi[b]
            idx_lo[olo:olo + len(lo_t)] = lo_t.astype(np.int16)
            idx_hi[ohi:ohi + len(hi_t)] = hi_t.astype(np.int16)
            cols_all[:, og:og + ll // 128] = _cols_wrap(lo_c.astype(np.float32), ll)
            cols_all[:, og + ll // 128:og + (ll + lh) // 128] = _cols_wrap(
                hi_c.astype(np.float32), lh)
            olo += ll
            ohi += lh
            og += (ll + lh) // 128
        flat = np.concatenate([idx_lo, idx_hi])
        idx_all = np.tile(flat.reshape(TI, 16).T, (8, 1))  # [128, TI]

        in_maps.append({
            "xt": xp, "w1t": w1t, "scwt": scwt, "gcnwt": gcnwt,
            "b1": vec(b1), "ln1_g": vec(ln1_g), "ln1_b": vec(ln1_b),
            "bn_g": vec(bn_g), "bn_b": vec(bn_b),
            "gcn_b": vec(gcn_b), "ln2_g": vec(ln2_g), "ln2_b": vec(ln2_b),
            "idx": idx_all, "cols": cols_all.astype(bfloat16), "dinvT": dvT,
            "iota_big": iota_big, "ident": ident, "ones": ones,
        })

    cfg = dict(P=P, N=N, NC=NC, NPAD=NPAD, NB=NB, D0=D0, D1=D1, D2=D2,
               TBL=TBL, SPLIT=split, TI=TI, TG=TG, Gmax=Gmax,
               TL_lo=TL_lo, TL_hi=TL_hi,
               qs=[int(v) for v in qs], qsz=[int(v) for v in qsz],
               qbase=[int(v) for v in qbase])
    return in_maps, L_lo, L_hi, cfg


def build_program(cfg, L_lo, L_hi, table_bf16=True):
    P, N = cfg["P"], cfg["N"]
    NPAD, NB = cfg["NPAD"], cfg["NB"]
    D0, D1, D2 = cfg["D0"], cfg["D1"], cfg["D2"]
    TBL, SPLIT, Gmax = cfg["TBL"], cfg["SPLIT"], cfg["Gmax"]
    TL_lo, TL_hi = cfg["TL_lo"], cfg["TL_hi"]
    qs, qsz, qbase = cfg["qs"], cfg["qsz"], cfg["qbase"]
    nq = len(qsz)
    KC, JC = D0 // 128, D1 // 128
    rg = [list(range(P))]
    TDT = BF16 if table_bf16 else F32R
    SUB = GATHER_SUB
    GPC = SUB // 128  # groups per gather call

    nc = bacc.Bacc("TRN2", target_bir_lowering=False, debug=False, num_devices=P,
                   num_swdge_queues=4)

    xt_d = nc.dram_tensor("xt", [NB, 128, KC, 128], BF16, kind="ExternalInput").ap()
    w1t_d = nc.dram_tensor("w1t", [D0, D1], BF16, kind="ExternalInput").ap()
    scwt_d = nc.dram_tensor("scwt", [D0, D1], BF16, kind="ExternalInput").ap()
    gcnwt_d = nc.dram_tensor("gcnwt", [D1, D2], BF16, kind="ExternalInput").ap()
    b1_d = nc.dram_tensor("b1", [1, D1], F32, kind="ExternalInput").ap()
    ln1g_d = nc.dram_tensor("ln1_g", [1, D1], F32, kind="ExternalInput").ap()
    ln1b_d = nc.dram_tensor("ln1_b", [1, D1], F32, kind="ExternalInput").ap()
    bng_d = nc.dram_tensor("bn_g", [1, D1], F32, kind="ExternalInput").ap()
    bnb_d = nc.dram_tensor("bn_b", [1, D1], F32, kind="ExternalInput").ap()
    gcnb_d = nc.dram_tensor("gcn_b", [1, D2], F32, kind="ExternalInput").ap()
    ln2g_d = nc.dram_tensor("ln2_g", [1, D2], F32, kind="ExternalInput").ap()
    ln2b_d = nc.dram_tensor("ln2_b", [1, D2], F32, kind="ExternalInput").ap()
    idx_d = nc.dram_tensor("idx", [128, cfg["TI"]], I16, kind="ExternalInput").ap()
    cols_d = nc.dram_tensor("cols", [128, cfg["TG"]], BF16, kind="ExternalInput").ap()
    dinv_d = nc.dram_tensor("dinvT", [128, NB], F32, kind="ExternalInput").ap()
    iotab_d = nc.dram_tensor("iota_big", [128, Gmax * 128], BF16, kind="ExternalInput").ap()
    ident_d = nc.dram_tensor("ident", [128, 128], BF16, kind="ExternalInput").ap()
    ones_d = nc.dram_tensor("ones", [128, 1], F32, kind="ExternalInput").ap()
    out_d = nc.dram_tensor("out", [NPAD, D2], F32, kind="ExternalOutput").ap()

    xwsq = [nc.dram_tensor(f"xwsq{q}", [qsz[q] * 128, D2], TDT) for q in range(nq)]
    table = nc.dram_tensor("table", [TBL, D2], TDT, addr_space="Shared")
    bn_in = nc.dram_tensor("bn_in", [1, 2 * D1], F32)
    bn_out = nc.dram_tensor("bn_out", [1, 2 * D1], F32, addr_space="Shared")
    ab_d = nc.dram_tensor("ab_d", [1, 2 * D1], F32)      # A|Ball bounce

    r = lambda ap: ap.bitcast(F32R)

    with tile.TileContext(nc) as tc, ExitStack() as ctx:
        const = ctx.enter_context(tc.tile_pool(name="const", bufs=1))

        def const_load(name, dram_ap, shape, dt=F32, bcast=False):
            t = const.tile(shape, dt, tag=name)
            src = dram_ap.to_broadcast(shape) if bcast else dram_ap
            nc.sync.dma_start(t[:], src)
            return t

        w1t_sb = const.tile([128, KC, D1], BF16, tag="w1t_sb")
        nc.sync.dma_start(w1t_sb[:], w1t_d.rearrange("(k p) n -> p k n", p=128))
        scwt_sb = const.tile([128, KC, D1], BF16, tag="scwt_sb")
        nc.sync.dma_start(scwt_sb[:], scwt_d.rearrange("(k p) n -> p k n", p=128))
        gcnwt_sb = const.tile([128, JC, D2], BF16, tag="gcnwt_sb")
        nc.sync.dma_start(gcnwt_sb[:], gcnwt_d.rearrange("(k p) n -> p k n", p=128))

        b1_t = const_load("b1_t", b1_d, [128, D1], bcast=True)
        gcnb_t = const_load("gcnb_t", gcnb_d, [128, D2], bcast=True)
        ln2g_t = const_load("ln2g_t", ln2g_d, [128, D2], bcast=True)
        ln2b_t = const_load("ln2b_t", ln2b_d, [128, D2], bcast=True)
        iotab_c = const_load("iotab_c", iotab_d, [128, Gmax * 128], dt=BF16)
        ident_sb = const_load("ident_sb", ident_d, [128, 128], dt=BF16)
        ones_sb = const_load("ones_sb", ones_d, [128, 1])
        dinv_sb = const_load("dinv_sb", dinv_d, [128, NB])
        ln1b_row = const_load("ln1b_row", ln1b_d, [1, D1])
        bng_row = const_load("bng_row", bng_d, [1, D1])
        bnb_row = const_load("bnb_row", bnb_d, [1, D1])
        ln1g_t = const_load("ln1g_t", ln1g_d, [128, D1], bcast=True)
        ln1g16 = const.tile([128, D1], BF16, tag="ln1g16")
        nc.vector.tensor_copy(ln1g16[:], ln1g_t[:])
        idx_all_sb = const.tile([128, cfg["TI"]], I16, tag="idx_all_sb")
        nc.sync.dma_start(idx_all_sb[:], idx_d[:])
        cols_all_sb = const.tile([128, cfg["TG"]], BF16, tag="cols_all_sb")
        nc.sync.dma_start(cols_all_sb[:], cols_d[:])

        eps_sb = const.tile([128, 1], F32, tag="eps_sb")
        nc.vector.memset(eps_sb[:], LN_EPS)
        eps1_sb = const.tile([1, 1], F32, tag="eps1_sb")
        nc.vector.memset(eps1_sb[:], BN_EPS)

        abt_t = const.tile([128, 2 * D1], F32, tag="abt_t")

        # constant-count registers for the gather calls
        lens = set()
        for tl in (TL_lo, TL_hi):
            if tl:
                lens.add(min(SUB, tl))
                if tl % SUB:
                    lens.add(tl % SUB)
        reg_of = {}
        for ln in sorted(lens):
            creg = ctx.enter_context(nc.gpsimd.register(name=f"cnt{ln}"))
            nc.gpsimd.reg_mov(creg, ln)
            reg_of[ln] = creg

        xt_v = xt_d  # [NB, 128, KC, 128] chunk-contiguous, bf16

        p12 = ExitStack()
        resid = p12.enter_context(tc.tile_pool(name="resid", bufs=1))
        fh_r = resid.tile([128, NB, D1], BF16, tag="fh_r", name="fh_r")
        ysc_r = resid.tile([128, NB, D1], BF16, tag="ysc_r", name="ysc_r")

        # ---- pass 1a: ysc = x @ scW.T ; BN sums via vector accumulation ---
        bn_sb = const.tile([1, 2 * D1], F32, tag="bn_sb")
        S_acc = const.tile([128, D1], F32, tag="S_acc")
        Q_acc = const.tile([128, D1], F32, tag="Q_acc")
        nc.vector.memset(S_acc[:], 0.0)
        nc.vector.memset(Q_acc[:], 0.0)
        with ExitStack() as p1:
            xpool = p1.enter_context(tc.tile_pool(name="xpool", bufs=3))
            work = p1.enter_context(tc.tile_pool(name="work1", bufs=3))
            ps = p1.enter_context(tc.tile_pool(name="ps1", bufs=2, space="PSUM"))

            for i in range(NB):
                xt = xpool.tile([128, KC, 128], BF16, tag="xt")
                nc.sync.dma_start(xt[:], xt_v[i])
                yscp = ps.tile([128, D1], F32, tag="yscp")
                for k in range(KC):
                    nc.tensor.matmul(yscp[:], xt[:, k, :], scwt_sb[:, k, :],
                                     start=(k == 0), stop=(k == KC - 1))
                ysc = ysc_r[:, i, :]
                nc.scalar.copy(ysc, yscp[:])
                sq = work.tile([128, D1], BF16, tag="sq")
                nc.gpsimd.tensor_mul(sq[:], ysc, ysc)
                nc.vector.tensor_add(S_acc[:], S_acc[:], yscp[:])
                nc.vector.tensor_add(Q_acc[:], Q_acc[:], sq[:])

            sum_ps = p1.enter_context(tc.tile_pool(name="sum_ps", bufs=1, space="PSUM"))
            sump = sum_ps.tile([1, D1], F32, tag="sump")
            sqsump = sum_ps.tile([1, D1], F32, tag="sqsump")
            nc.tensor.matmul(sump[:], ones_sb[:], S_acc[:],
                             start=True, stop=True)
            nc.tensor.matmul(sqsump[:], ones_sb[:], Q_acc[:],
                             start=True, stop=True)
            nc.vector.tensor_copy(bn_sb[:, 0:D1], sump[:])
            nc.vector.tensor_copy(bn_sb[:, D1:2 * D1], sqsump[:])

        # kick the BatchNorm AllReduce; it overlaps pass 1b
        nc.scalar.dma_start(bn_in.ap()[:], bn_sb[:])
        nc.gpsimd.collective_compute(
            "AllReduce", mybir.AluOpType.add, replica_groups=rg,
            ins=[bn_in.ap()[:]], outs=[bn_out.ap()[:]])

        # ---- pass 1b: y1 = x @ W1.T ; LayerNorm1 (overlaps the AllReduce) --
        with ExitStack() as p1b:
            xpool = p1b.enter_context(tc.tile_pool(name="xpool1b", bufs=3))
            work = p1b.enter_context(tc.tile_pool(name="work1b", bufs=3))
            ps = p1b.enter_context(tc.tile_pool(name="ps1b", bufs=2, space="PSUM"))

            for i in range(NB):
                xt = xpool.tile([128, KC, 128], BF16, tag="xt")
                nc.sync.dma_start(xt[:], xt_v[i])
                y1p = ps.tile([128, D1], F32, tag="y1p")
                for k in range(KC):
                    nc.tensor.matmul(y1p[:], xt[:, k, :], w1t_sb[:, k, :],
                                     start=(k == 0), stop=(k == KC - 1))
                y1b = work.tile([128, D1], F32, tag="y1b")
                nc.vector.tensor_add(y1b[:], y1p[:], b1_t[:])
                st = work.tile([128, 6], F32, tag="st")
                nc.vector.bn_stats(st[:], y1b[:])
                mv = work.tile([128, 2], F32, tag="mv")
                nc.vector.bn_aggr(mv[:], st[:])
                if USE_RSQRT:
                    nc.scalar.activation(mv[:, 1:2], mv[:, 1:2],
                                         mybir.ActivationFunctionType.Rsqrt,
                                         bias=eps_sb[:])
                else:
                    nc.scalar.activation(mv[:, 1:2], mv[:, 1:2],
                                         mybir.ActivationFunctionType.Sqrt,
                                         bias=eps_sb[:])
                    nc.vector.reciprocal(mv[:, 1:2], mv[:, 1:2])
                nmr = work.tile([128, 1], F32, tag="nmr")
                nc.vector.tensor_scalar(nmr[:], mv[:, 0:1], mv[:, 1:2], -1.0,
                                        op0=mybir.AluOpType.mult,
                                        op1=mybir.AluOpType.mult)
                f = work.tile([128, D1], BF16, tag="f")
                nc.scalar.activation(f[:], y1b[:],
                                     mybir.ActivationFunctionType.Identity,
                                     bias=nmr[:], scale=mv[:, 1:2])
                nc.gpsimd.tensor_mul(fh_r[:, i, :], f[:], ln1g16[:])

        # ---- BatchNorm stats: read AR result, form A/Ball vectors ---------
        bnall = const.tile([1, 2 * D1], F32, tag="bnall")
        nc.scalar.dma_start(bnall[:], bn_out.ap()[:])

        mean_r = const.tile([1, D1], F32, tag="mean_r")
        nc.scalar.mul(mean_r[:], bnall[:, 0:D1], 1.0 / N)
        var_r = const.tile([1, D1], F32, tag="var_r")
        nc.scalar.mul(var_r[:], bnall[:, D1:2 * D1], 1.0 / N)
        msq = const.tile([1, D1], F32, tag="msq")
        nc.vector.tensor_mul(msq[:], mean_r[:], mean_r[:])
        nc.vector.tensor_sub(var_r[:], var_r[:], msq[:])
        nc.scalar.activation(var_r[:], var_r[:],
                             mybir.ActivationFunctionType.Sqrt, bias=eps1_sb[:])
        nc.vector.reciprocal(var_r[:], var_r[:])          # rstd
        ab_row = const.tile([1, 2 * D1], F32, tag="ab_row")
        A_row = ab_row[:, 0:D1]
        ball_row = ab_row[:, D1:2 * D1]
        nc.vector.tensor_mul(A_row, var_r[:], bng_row[:])
        mA = const.tile([1, D1], F32, tag="mA")
        nc.vector.tensor_mul(mA[:], mean_r[:], A_row)
        nc.vector.tensor_sub(ball_row, bnb_row[:], mA[:])
        nc.vector.tensor_add(ball_row, ball_row, ln1b_row[:])
        nc.scalar.dma_start(ab_d.ap()[0:1, :], ab_row[:])
        nc.scalar.dma_start(abt_t[:], ab_d.ap()[0:1, :].to_broadcast([128, 2 * D1]))
        abt16 = const.tile([128, 2 * D1], BF16, tag="abt16")
        nc.vector.tensor_copy(abt16[:], abt_t[:])
        A16 = abt16[:, 0:D1]
        Ball16 = abt16[:, D1:2 * D1]

        # ---- pass 2: h, hT, xw, scale, store (+chunked AG) ----------------
        with ExitStack() as p2:
            work = p2.enter_context(tc.tile_pool(name="work2", bufs=6))
            ps = p2.enter_context(tc.tile_pool(name="ps2", bufs=2, space="PSUM"))
            tps = p2.enter_context(tc.tile_pool(name="tps", bufs=3, space="PSUM"))

            q_cur = 0
            for i in range(NB):
                t = work.tile([128, D1], BF16, tag="t")
                nc.gpsimd.tensor_mul(t[:], ysc_r[:, i, :], A16)
                f = work.tile([128, D1], BF16, tag="f")
                nc.vector.tensor_add(f[:], t[:], fh_r[:, i, :])
                nc.vector.tensor_add(f[:], f[:], Ball16)
                h = work.tile([128, D1], BF16, tag="h")
                if USE_LRELU:
                    nc.scalar.activation(h[:], f[:],
                                         mybir.ActivationFunctionType.Lrelu,
                                         alpha=SLOPE)
                else:
                    hl = work.tile([128, D1], BF16, tag="hl")
                    nc.scalar.mul(hl[:], f[:], SLOPE)
                    nc.vector.tensor_max(h[:], f[:], hl[:])

                ht = work.tile([128, JC, 128], BF16, tag="ht")
                for j in range(JC):
                    tp = tps.tile([128, 128], BF16, tag="tp")
                    nc.tensor.transpose(tp[:], h[:, j * 128:(j + 1) * 128], ident_sb[:])
                    if j % 2 == 0:
                        nc.scalar.copy(ht[:, j, :], tp[:])
                    else:
                        nc.vector.tensor_copy(ht[:, j, :], tp[:])
                xwp = ps.tile([128, D2], F32, tag="xwp")
                for j in range(JC):
                    nc.tensor.matmul(xwp[:], ht[:, j, :], gcnwt_sb[:, j, :],
                                     start=(j == 0), stop=(j == JC - 1))
                xws = work.tile([128, D2], TDT, tag="xws")
                nc.vector.tensor_scalar(xws[:], xwp[:], dinv_sb[:, i:i + 1], None,
                                        op0=mybir.AluOpType.mult)
                q = q_cur
                nc.sync.dma_start(
                    xwsq[q].ap()[(i - qs[q]) * 128:(i - qs[q] + 1) * 128, :],
                    xws[:])
                if i + 1 == qs[q_cur + 1]:
                    # quarter complete: AllGather it into its table slice
                    nc.gpsimd.collective_compute(
                        "AllGather", mybir.AluOpType.bypass, replica_groups=rg,
                        ins=[xwsq[q].ap()[:]],
                        outs=[table.ap()[qbase[q]:qbase[q + 1], :]])
                    q_cur += 1

        p12.close()

        # ---- pass 3: gather + S-matmul aggregation + LN2 -----------------
        with ExitStack() as p3:
            mpool = p3.enter_context(tc.tile_pool(name="mpool", bufs=MSG_RING))
            spool = p3.enter_context(tc.tile_pool(name="spool", bufs=3))
            work = p3.enter_context(tc.tile_pool(name="work3", bufs=3))
            ps = p3.enter_context(tc.tile_pool(name="ps3", bufs=4, space="PSUM"))

            # call table: stream-packed gather calls; groups are 128-aligned
            # within calls, so each matmul group maps to one call slice.
            # stream "lo": groups [0, TL_lo/128); "hi": [TL_lo/128, TG)
            calls = []  # (stream, idx16_off, n_idx, first_group)
            for stream, tl, base_g, base_i in (
                    ("lo", TL_lo, 0, 0),
                    ("hi", TL_hi, TL_lo // 128, TL_lo // 16)):
                o = 0
                while o < tl:
                    n = min(SUB, tl - o)
                    calls.append((stream, base_i + o // 16, n, base_g + o // 128))
                    o += n
            n_lo_calls = sum(1 for c in calls if c[0] == "lo")

            call_tiles = {}
            emitted = {"lo": 0, "hi": 0}
            qrr = [0]
            tbl_lo = table.ap()[0:SPLIT, :]
            tbl_hi = table.ap()[SPLIT:TBL, :]

            def emit_call(ci):
                stream, ioff, n, g0 = calls[ci]
                mt = mpool.tile([128, GPC, D2], TDT, tag="msg")
                nc.gpsimd.dma_gather(
                    out_ap=mt[:, 0:n // 128, :],
                    in_ap=tbl_lo if stream == "lo" else tbl_hi,
                    idxs_ap=idx_all_sb[:, ioff:ioff + n // 16],
                    num_idxs=n, num_idxs_reg=reg_of[n], elem_size=D2,
                    single_packet=True, queue_num=qrr[0] % 4)
                qrr[0] += 1
                call_tiles[ci] = mt

            def group_view(g):
                """global group id -> [128, D2] subview of its call tile.

                Calls are emitted lazily per stream, in consumption order, so
                msg-ring WAR edges always point backwards (no cycles)."""
                if g < TL_lo // 128:
                    stream, base, ci_l = "lo", 0, g // GPC
                    goff = g - ci_l * GPC
                else:
                    gh = g - TL_lo // 128
                    stream, base, ci_l = "hi", n_lo_calls, gh // GPC
                    goff = gh - ci_l * GPC
                while emitted[stream] <= ci_l:
                    emit_call(base + emitted[stream])
                    emitted[stream] += 1
                return call_tiles[base + ci_l][:, goff, :]

            og = 0
            for b in range(NB):
                G = (L_lo[b] + L_hi[b]) // 128
                glist = []
                lo_g0 = sum(L_lo[:b]) // 128
                hi_g0 = TL_lo // 128 + sum(L_hi[:b]) // 128
                glist += [lo_g0 + k for k in range(L_lo[b] // 128)]
                glist += [hi_g0 + k for k in range(L_hi[b] // 128)]

                # the block's own scaled rows (self-loop term), re-read from
                # the quarter scratch written in pass 2
                qb = next(q for q in range(nq) if qs[q] <= b < qs[q + 1])
                xwsb = work.tile([128, D2], TDT, tag="xwsb")
                nc.sync.dma_start(
                    xwsb[:], xwsq[qb].ap()[(b - qs[qb]) * 128:(b - qs[qb] + 1) * 128, :])

                cols_sb = cols_all_sb[:, og:og + G]
                S_all = spool.tile([128, G, 128], TDT, tag="S_all")
                i3 = iotab_c[:, 0:G * 128].rearrange("p (g t) -> p g t", g=G)
                cb = cols_sb[:, 0:G].unsqueeze(2).broadcast_to((128, G, 128))
                if ISEQ_SWAP:
                    nc.vector.tensor_tensor(out=S_all[:], in0=cb, in1=i3,
                                            op=mybir.AluOpType.is_equal)
                else:
                    nc.vector.tensor_tensor(out=S_all[:], in0=i3, in1=cb,
                                            op=mybir.AluOpType.is_equal)

                acc = ps.tile([128, D2], F32, tag="acc")
                for k, g in enumerate(glist):
                    mv_view = group_view(g)
                    nc.tensor.matmul(acc[:], S_all[:, k, :], mv_view,
                                     start=(k == 0), stop=False)
                # self-loop: acc[t,:] += xws[t,:] via identity matmul
                if table_bf16:
                    nc.tensor.matmul(acc[:], ident_sb[:], xwsb[:],
                                     start=False, stop=True)
                else:
                    nc.tensor.matmul(acc[:], ident_sb[:].bitcast(F32R), xwsb[:],
                                     start=False, stop=True)

                ev = work.tile([128, D2], F32, tag="ev")
                nc.vector.tensor_scalar(ev[:], acc[:], dinv_sb[:, b:b + 1], None,
                                        op0=mybir.AluOpType.mult)
                nc.vector.tensor_add(ev[:], ev[:], gcnb_t[:])
                st = work.tile([128, 6], F32, tag="st3")
                nc.vector.bn_stats(st[:], ev[:])
                mv = work.tile([128, 2], F32, tag="mv3")
                nc.vector.bn_aggr(mv[:], st[:])
                nc.scalar.activation(mv[:, 1:2], mv[:, 1:2],
                                     mybir.ActivationFunctionType.Sqrt,
                                     bias=eps_sb[:])
                nc.vector.reciprocal(mv[:, 1:2], mv[:, 1:2])
                nmr = work.tile([128, 1], F32, tag="nmr3")
                nc.vector.tensor_scalar(nmr[:], mv[:, 0:1], mv[:, 1:2], -1.0,
                                        op0=mybir.AluOpType.mult,
                                        op1=mybir.AluOpType.mult)
                f2 = work.tile([128, D2], F32, tag="f2")
                nc.scalar.activation(f2[:], ev[:],
                                     mybir.ActivationFunctionType.Identity,
                                     bias=nmr[:], scale=mv[:, 1:2])
                nc.vector.tensor_mul(f2[:], f2[:], ln2g_t[:])
                nc.vector.tensor_add(f2[:], f2[:], ln2b_t[:])
                # leaky via mul+max: Lrelu here would thrash the act table
                # against Sqrt/Identity every block
                oo = work.tile([128, D2], F32, tag="oo")
                ol = work.tile([128, D2], F32, tag="ol")
                nc.scalar.mul(ol[:], f2[:], SLOPE)
                nc.vector.tensor_max(oo[:], f2[:], ol[:])
                nc.sync.dma_start(out_d[b * 128:(b + 1) * 128, :], oo[:])
                og += G

    nc.compile()
    return nc


_last_results = None


def kernel(**inputs) -> np.ndarray:
    global _last_results
    in_maps, L_lo, L_hi, cfg = preprocess(**inputs)
    table_bf16 = os.environ.get("GNN_TABLE_FP32", "") != "1"
    nc = build_program(cfg, L_lo, L_hi, table_bf16=table_bf16)
    trace = os.environ.get("GNN_TRACE", "") == "1"
    res = run_bass_kernel_spmd(nc, in_maps, core_ids=list(range(cfg["P"])),
                               trace=trace)
    _last_results = res
    NC = cfg["NC"]
    return np.concatenate([res.results[c]["out"][:NC] for c in range(cfg["P"])],
                          axis=0)


# revision 13
# speedup vs baseline: 1.0295x; 1.0295x over previous
"""Trainium2 Bass kernel for nn_ATAC_Encoder (GCN message passing), 8 cores.

Math (reference):
    f  = LayerNorm(x @ W1.T + b1) * ln1_g + ln1_b
    sc = BatchNorm(x @ sc_W.T + sc_b) * bn_g + bn_b      (batch stats over nodes)
    h  = leaky_relu(f + sc, 0.01)
    g  = GCNConv(h, edge_index, gcn_W, gcn_b)            (sym-norm, self-loops)
    out = leaky_relu(LayerNorm(g) * ln2_g + ln2_b, 0.01)

Distribution: nodes block-sharded across 8 NeuronCores (6250 each, padded to
6272 = 49*128). Pipeline per core:
  pass1a: ysc = x @ scW.T, BatchNorm partial sums accumulated on the vector
          engine (S_acc/Q_acc) then reduced with two f32r ones-matmuls; the
          2KB AllReduce is kicked immediately so it overlaps pass1b.
  pass1b: y1 = x @ W1.T (x re-streamed from HBM), LayerNorm per block.
  pass2:  h = leaky_relu(ysc*A + f + Ball), hT via PE transpose, xw = hT @
          gcnW.T, scale by dinv, store; the scaled table is AllGathered in 4
          node-quarter chunks (small quarter first so the collective chain
          starts early).
  pass3:  per-edge source rows are gathered from the table with SWDGE
          dma_gather calls packed into fixed-size index streams (one lo / one
          hi stream per core, int16 table-half split), with gather counts in
          preloaded constant registers (no per-call register loads);
          aggregation via selector-matrix matmuls on the tensor engine:
          acc[t,:] += S_g.T @ msgs_g, then dinv scale, +gcn_b, LayerNorm,
          leaky_relu.
"""

import os
import numpy as np
from contextlib import ExitStack

import concourse.bass as bass
import concourse.mybir as mybir
import concourse.tile as tile
from concourse import bacc
from concourse.bass_utils import run_bass_kernel_spmd

F32 = mybir.dt.float32
F32R = mybir.dt.float32r
BF16 = mybir.dt.bfloat16
I16 = mybir.dt.int16

LN_EPS = 1e-5
BN_EPS = 1e-5
SLOPE = 0.01
GATHER_SUB = int(os.environ.get("GNN_SUB", "896"))  # idxs per gather call
MSG_RING = int(os.environ.get("GNN_RING", "20"))    # gather ring slots
USE_LRELU = os.environ.get("GNN_NO_LRELU", "") != "1"  # pass-2 only (act table)
USE_RSQRT = os.environ.get("GNN_RSQRT", "") == "1"  # blocked: accuracy issues
ISEQ_SWAP = os.environ.get("GNN_ISEQ_SWAP", "") == "1"
QS_PATTERN = os.environ.get("GNN_QS", "4,13,15,17")


def _cols_wrap(v: np.ndarray, L: int) -> np.ndarray:
    """[cnt] col list -> [128, L/128] fp32 (slot m -> [m%128, m//128]), pad -1."""
    a = np.full(L, -1.0, np.float32)
    a[: len(v)] = v
    return a.reshape(L // 128, 128).T


def preprocess(x, edge_index, W1, b1, ln1_g, ln1_b, sc_W, sc_b, bn_g, bn_b,
               gcn_W, gcn_b, ln2_g, ln2_b, n_cores=8, split=32768):
    """Shard inputs; returns (in_maps, L_lo, L_hi, cfg)."""
    x = np.asarray(x, np.float32)
    ei = np.asarray(edge_index)
    N, D0 = x.shape
    D1 = W1.shape[0]
    D2 = gcn_W.shape[0]
    P = n_cores
    NC = N // P
    NPAD = ((NC + 127) // 128) * 128
    NB = NPAD // 128
    TBL = P * NPAD

    # node-quarter chunking of the AllGather; table rows quarter-major then
    # core-major. Small first quarter so the AG chain starts early.
    szs = [int(s) for s in QS_PATTERN.split(",")]
    assert sum(szs) == NB, (szs, NB)
    nq = len(szs)
    qs = [0]
    for z in szs:
        qs.append(qs[-1] + z)
    qsz = np.array(szs)
    qbase = np.cumsum([0] + [P * int(s) * 128 for s in qsz])
    q_of_block = np.repeat(np.arange(nq), qsz)
    qs_arr = np.array(qs[:nq])

    # self-loops are handled locally in pass 3 (identity matmul of the
    # block's own xws rows); only real edges go through the gather. The
    # degree still counts the self-loop (A+I normalization).
    loops = np.arange(N, dtype=np.int64)
    row = ei[0].astype(np.int64)
    col = ei[1].astype(np.int64)

    deg = np.bincount(np.concatenate([col, loops]), minlength=N).astype(np.float64)
    dinv = (1.0 / np.sqrt(deg)).astype(np.float32)  # deg >= 1 (self-loops)

    sown = row // NC
    sloc = row - sown * NC
    sblk = sloc >> 7
    sq = q_of_block[sblk]
    trow = qbase[sq] + sown * (qsz[sq] * 128) + (sloc - qs_arr[sq] * 128)
    owner = col // NC
    lcol = col - owner * NC

    # bucket edges per (core, target block, lo/hi table half)
    per = []
    for c in range(P):
        m = owner == c
        tr, lc = trow[m], lcol[m]
        blk = lc >> 7
        hi = (tr >= split).astype(np.int64)
        order = np.lexsort((tr, hi, blk))
        tr, lc, blk, hi = tr[order], lc[order], blk[order], hi[order]
        bounds = np.searchsorted(blk * 2 + hi, np.arange(2 * NB + 1))
        blocks = []
        for b in range(NB):
            lo_s, lo_e = bounds[2 * b], bounds[2 * b + 1]
            hi_s, hi_e = bounds[2 * b + 1], bounds[2 * b + 2]
            blocks.append((tr[lo_s:lo_e], tr[hi_s:hi_e] - split,
                           lc[lo_s:lo_e] - b * 128, lc[hi_s:hi_e] - b * 128))
        per.append(blocks)

    def rup128(n):
        return ((n + 127) // 128) * 128

    L_lo = [rup128(max(len(per[c][b][0]) for c in range(P))) for b in range(NB)]
    L_hi = [rup128(max(len(per[c][b][1]) for c in range(P))) for b in range(NB)]
    TL_lo = sum(L_lo)
    TL_hi = sum(L_hi)

    TI = (TL_lo + TL_hi) // 16
    TG = (TL_lo + TL_hi) // 128

    from ml_dtypes import bfloat16

    KC0 = D0 // 128
    w1t = np.ascontiguousarray(W1.T).astype(bfloat16)      # [D0, D1]
    scwt = np.ascontiguousarray(sc_W.T).astype(bfloat16)   # [D0, D1]
    gcnwt = np.ascontiguousarray(gcn_W.T).astype(bfloat16)  # [D1, D2]
    Gmax = max((L_lo[b] + L_hi[b]) // 128 for b in range(NB))
    iota = np.broadcast_to(np.arange(128, dtype=np.float32), (128, 128))
    iota_big = np.ascontiguousarray(np.tile(iota, (1, Gmax))).astype(bfloat16)
    ident = np.eye(128, dtype=np.float32).astype(bfloat16)
    ones = np.ones((128, 1), np.float32)

    vec = lambda v: np.asarray(v, np.float32).reshape(1, -1)

    in_maps = []
    for c in range(P):
        xp = np.zeros((D0, NPAD), np.float32)
        xp[:, :NC] = x[c * NC:(c + 1) * NC].T
        # pretile: [NB, p(d0 in k-chunk), k, n] so each chunk is one
        # contiguous [128, KC*128] DMA
        xp = np.ascontiguousarray(
            xp.reshape(KC0, 128, NB, 128).transpose(2, 1, 0, 3)).astype(bfloat16)

        dv = np.zeros(NPAD, np.float32)
        dv[:NC] = dinv[c * NC:(c + 1) * NC]
        dvT = np.ascontiguousarray(dv.reshape(NB, 128).T)  # [128, NB]

        # flat per-half index streams (block-major, each block 128-padded,
        # pad slots gather row 0) + cols (pad -1 -> S column of zeros)
        idx_lo = np.zeros(TL_lo, np.int16)
        idx_hi = np.zeros(TL_hi, np.int16)
        cols_all = np.full((128, TG), -1.0, np.float32)
        olo = ohi = og = 0
        for b in range(NB):
            lo_t, hi_t, lo_c, hi_c = per[c][b]
            ll, lh = L_lo[b], L_hi[b]
            idx_lo[olo:olo + len(lo_t)] = lo_t.astype(np.int16)
            idx_hi[ohi:ohi + len(hi_t)] = hi_t.astype(np.int16)
            cols_all[:, og:og + ll // 128] = _cols_wrap(lo_c.astype(np.float32), ll)
            cols_all[:, og + ll // 128:og + (ll + lh) // 128] = _cols_wrap(
                hi_c.astype(np.float32), lh)
            olo += ll
            ohi += lh
            og += (ll + lh) // 128
        flat = np.concatenate([idx_lo, idx_hi])
        idx_all = np.tile(flat.reshape(TI, 16).T, (8, 1))  # [128, TI]

        in_maps.append({
            "xt": xp, "w1t": w1t, "scwt": scwt, "gcnwt": gcnwt,
            "b1": vec(b1), "ln1_g": vec(ln1_g), "ln1_b": vec(ln1_b),
            "bn_g": vec(bn_g), "bn_b": vec(bn_b),
            "gcn_b": vec(gcn_b), "ln2_g": vec(ln2_g), "ln2_b": vec(ln2_b),
            "idx": idx_all, "cols": cols_all.astype(bfloat16), "dinvT": dvT,
            "iota_big": iota_big, "ident": ident, "ones": ones,
        })

    cfg = dict(P=P, N=N, NC=NC, NPAD=NPAD, NB=NB, D0=D0, D1=D1, D2=D2,
               TBL=TBL, SPLIT=split, TI=TI, TG=TG, Gmax=Gmax,
               TL_lo=TL_lo, TL_hi=TL_hi,
               qs=[int(v) for v in qs], qsz=[int(v) for v in qsz],
               qbase=[int(v) for v in qbase])
    return in_maps, L_lo, L_hi, cfg


def build_program(cfg, L_lo, L_hi, table_bf16=True):
    P, N = cfg["P"], cfg["N"]
    NPAD, NB = cfg["NPAD"], cfg["NB"]
    D0, D1, D2 = cfg["D0"], cfg["D1"], cfg["D2"]
    TBL, SPLIT, Gmax = cfg["TBL"], cfg["SPLIT"], cfg["Gmax"]
    TL_lo, TL_hi = cfg["TL_lo"], cfg["TL_hi"]
    qs, qsz, qbase = cfg["qs"], cfg["qsz"], cfg["qbase"]
    nq = len(qsz)
    KC, JC = D0 // 128, D1 // 128
    rg = [list(range(P))]
    TDT = BF16 if table_bf16 else F32R
    SUB = GATHER_SUB
    GPC = SUB // 128  # groups per gather call

    nc = bacc.Bacc("TRN2", target_bir_lowering=False, debug=False, num_devices=P,
                   num_swdge_queues=4)

    xt_d = nc.dram_tensor("xt", [NB, 128, KC, 128], BF16, kind="ExternalInput").ap()
    w1t_d = nc.dram_tensor("w1t", [D0, D1], BF16, kind="ExternalInput").ap()
    scwt_d = nc.dram_tensor("scwt", [D0, D1], BF16, kind="ExternalInput").ap()
    gcnwt_d = nc.dram_tensor("gcnwt", [D1, D2], BF16, kind="ExternalInput").ap()
    b1_d = nc.dram_tensor("b1", [1, D1], F32, kind="ExternalInput").ap()
    ln1g_d = nc.dram_tensor("ln1_g", [1, D1], F32, kind="ExternalInput").ap()
    ln1b_d = nc.dram_tensor("ln1_b", [1, D1], F32, kind="ExternalInput").ap()
    bng_d = nc.dram_tensor("bn_g", [1, D1], F32, kind="ExternalInput").ap()
    bnb_d = nc.dram_tensor("bn_b", [1, D1], F32, kind="ExternalInput").ap()
    gcnb_d = nc.dram_tensor("gcn_b", [1, D2], F32, kind="ExternalInput").ap()
    ln2g_d = nc.dram_tensor("ln2_g", [1, D2], F32, kind="ExternalInput").ap()
    ln2b_d = nc.dram_tensor("ln2_b", [1, D2], F32, kind="ExternalInput").ap()
    idx_d = nc.dram_tensor("idx", [128, cfg["TI"]], I16, kind="ExternalInput").ap()
    cols_d = nc.dram_tensor("cols", [128, cfg["TG"]], BF16, kind="ExternalInput").ap()
    dinv_d = nc.dram_tensor("dinvT", [128, NB], F32, kind="ExternalInput").ap()
    iotab_d = nc.dram_tensor("iota_big", [128, Gmax * 128], BF16, kind="ExternalInput").ap()
    ident_d = nc.dram_tensor("ident", [128, 128], BF16, kind="ExternalInput").ap()
    ones_d = nc.dram_tensor("ones", [128, 1], F32, kind="ExternalInput").ap()
    out_d = nc.dram_tensor("out", [NPAD, D2], F32, kind="ExternalOutput").ap()

    xwsq = [nc.dram_tensor(f"xwsq{q}", [qsz[q] * 128, D2], TDT) for q in range(nq)]
    table = nc.dram_tensor("table", [TBL, D2], TDT, addr_space="Shared")
    bn_in = nc.dram_tensor("bn_in", [1, 2 * D1], F32)
    bn_out = nc.dram_tensor("bn_out", [1, 2 * D1], F32, addr_space="Shared")
    ab_d = nc.dram_tensor("ab_d", [1, 2 * D1], F32)      # A|Ball bounce

    r = lambda ap: ap.bitcast(F32R)

    with tile.TileContext(nc) as tc, ExitStack() as ctx:
        const = ctx.enter_context(tc.tile_pool(name="const", bufs=1))

        def const_load(name, dram_ap, shape, dt=F32, bcast=False):
            t = const.tile(shape, dt, tag=name)
            src = dram_ap.to_broadcast(shape) if bcast else dram_ap
            nc.sync.dma_start(t[:], src)
            return t

        w1t_sb = const.tile([128, KC, D1], BF16, tag="w1t_sb")
        nc.sync.dma_start(w1t_sb[:], w1t_d.rearrange("(k p) n -> p k n", p=128))
        scwt_sb = const.tile([128, KC, D1], BF16, tag="scwt_sb")
        nc.sync.dma_start(scwt_sb[:], scwt_d.rearrange("(k p) n -> p k n", p=128))
        gcnwt_sb = const.tile([128, JC, D2], BF16, tag="gcnwt_sb")
        nc.sync.dma_start(gcnwt_sb[:], gcnwt_d.rearrange("(k p) n -> p k n", p=128))

        b1_t = const_load("b1_t", b1_d, [128, D1], bcast=True)
        gcnb_t = const_load("gcnb_t", gcnb_d, [128, D2], bcast=True)
        ln2g_t = const_load("ln2g_t", ln2g_d, [128, D2], bcast=True)
        ln2b_t = const_load("ln2b_t", ln2b_d, [128, D2], bcast=True)
        iotab_c = const_load("iotab_c", iotab_d, [128, Gmax * 128], dt=BF16)
        ident_sb = const_load("ident_sb", ident_d, [128, 128], dt=BF16)
        ones_sb = const_load("ones_sb", ones_d, [128, 1])
        dinv_sb = const_load("dinv_sb", dinv_d, [128, NB])
        ln1b_row = const_load("ln1b_row", ln1b_d, [1, D1])
        bng_row = const_load("bng_row", bng_d, [1, D1])
        bnb_row = const_load("bnb_row", bnb_d, [1, D1])
        ln1g_t = const_load("ln1g_t", ln1g_d, [128, D1], bcast=True)
        ln1g16 = const.tile([128, D1], BF16, tag="ln1g16")
        nc.vector.tensor_copy(ln1g16[:], ln1g_t[:])
        idx_all_sb = const.tile([128, cfg["TI"]], I16, tag="idx_all_sb")
        nc.sync.dma_start(idx_all_sb[:], idx_d[:])
        cols_all_sb = const.tile([128, cfg["TG"]], BF16, tag="cols_all_sb")
        nc.sync.dma_start(cols_all_sb[:], cols_d[:])

        eps_sb = const.tile([128, 1], F32, tag="eps_sb")
        nc.vector.memset(eps_sb[:], LN_EPS)
        eps1_sb = const.tile([1, 1], F32, tag="eps1_sb")
        nc.vector.memset(eps1_sb[:], BN_EPS)

        abt_t = const.tile([128, 2 * D1], F32, tag="abt_t")

        # constant-count registers for the gather calls
        lens = set()
        for tl in (TL_lo, TL_hi):
            if tl:
                lens.add(min(SUB, tl))
                if tl % SUB:
                    lens.add(tl % SUB)
        reg_of = {}
        for ln in sorted(lens):
            creg = ctx.enter_context(nc.gpsimd.register(name=f"cnt{ln}"))
            nc.gpsimd.reg_mov(creg, ln)
            reg_of[ln] = creg

        xt_v = xt_d  # [NB, 128, KC, 128] chunk-contiguous, bf16

        p12 = ExitStack()
        resid = p12.enter_context(tc.tile_pool(name="resid", bufs=1))
        fh_r = resid.tile([128, NB, D1], BF16, tag="fh_r", name="fh_r")
        ysc_r = resid.tile([128, NB, D1], BF16, tag="ysc_r", name="ysc_r")

        # ---- pass 1a: ysc = x @ scW.T ; BN sums via vector accumulation ---
        bn_sb = const.tile([1, 2 * D1], F32, tag="bn_sb")
        S_acc = const.tile([128, D1], F32, tag="S_acc")
        Q_acc = const.tile([128, D1], F32, tag="Q_acc")
        nc.vector.memset(S_acc[:], 0.0)
        nc.vector.memset(Q_acc[:], 0.0)
        with ExitStack() as p1:
            xpool = p1.enter_context(tc.tile_pool(name="xpool", bufs=3))
            work = p1.enter_context(tc.tile_pool(name="work1", bufs=3))
            ps = p1.enter_context(tc.tile_pool(name="ps1", bufs=2, space="PSUM"))

            for i in range(NB):
                xt = xpool.tile([128, KC, 128], BF16, tag="xt")
                nc.sync.dma_start(xt[:], xt_v[i])
                yscp = ps.tile([128, D1], F32, tag="yscp")
                for k in range(KC):
                    nc.tensor.matmul(yscp[:], xt[:, k, :], scwt_sb[:, k, :],
                                     start=(k == 0), stop=(k == KC - 1))
                ysc = ysc_r[:, i, :]
                nc.scalar.copy(ysc, yscp[:])
                sq = work.tile([128, D1], BF16, tag="sq")
                nc.gpsimd.tensor_mul(sq[:], ysc, ysc)
                nc.vector.tensor_add(S_acc[:], S_acc[:], yscp[:])
                nc.vector.tensor_add(Q_acc[:], Q_acc[:], sq[:])

            sum_ps = p1.enter_context(tc.tile_pool(name="sum_ps", bufs=1, space="PSUM"))
            sump = sum_ps.tile([1, D1], F32, tag="sump")
            sqsump = sum_ps.tile([1, D1], F32, tag="sqsump")
            nc.tensor.matmul(sump[:], ones_sb[:], S_acc[:],
                             start=True, stop=True)
            nc.tensor.matmul(sqsump[:], ones_sb[:], Q_acc[:],
                             start=True, stop=True)
            nc.vector.tensor_copy(bn_sb[:, 0:D1], sump[:])
            nc.vector.tensor_copy(bn_sb[:, D1:2 * D1], sqsump[:])

        # kick the BatchNorm AllReduce; it overlaps pass 1b
        nc.scalar.dma_start(bn_in.ap()[:], bn_sb[:])
        nc.gpsimd.collective_compute(
            "AllReduce", mybir.AluOpType.add, replica_groups=rg,
            ins=[bn_in.ap()[:]], outs=[bn_out.ap()[:]])

        # ---- pass 1b: y1 = x @ W1.T ; LayerNorm1 (overlaps the AllReduce) --
        with ExitStack() as p1b:
            xpool = p1b.enter_context(tc.tile_pool(name="xpool1b", bufs=3))
            work = p1b.enter_context(tc.tile_pool(name="work1b", bufs=3))
            ps = p1b.enter_context(tc.tile_pool(name="ps1b", bufs=2, space="PSUM"))

            for i in range(NB):
                xt = xpool.tile([128, KC, 128], BF16, tag="xt")
                nc.sync.dma_start(xt[:], xt_v[i])
                y1p = ps.tile([128, D1], F32, tag="y1p")
                for k in range(KC):
                    nc.tensor.matmul(y1p[:], xt[:, k, :], w1t_sb[:, k, :],
                                     start=(k == 0), stop=(k == KC - 1))
                y1b = work.tile([128, D1], F32, tag="y1b")
                nc.vector.tensor_add(y1b[:], y1p[:], b1_t[:])
                st = work.tile([128, 6], F32, tag="st")
                nc.vector.bn_stats(st[:], y1b[:])
                mv = work.tile([128, 2], F32, tag="mv")
                nc.vector.bn_aggr(mv[:], st[:])
                if USE_RSQRT:
                    nc.scalar.activation(mv[:, 1:2], mv[:, 1:2],
                                         mybir.ActivationFunctionType.Rsqrt,
                                         bias=eps_sb[:])
                else:
                    nc.scalar.activation(mv[:, 1:2], mv[:, 1:2],
                                         mybir.ActivationFunctionType.Sqrt,
                                         bias=eps_sb[:])
                    nc.vector.reciprocal(mv[:, 1:2], mv[:, 1:2])
                nmr = work.tile([128, 1], F32, tag="nmr")
                nc.vector.tensor_scalar(nmr[:], mv[:, 0:1], mv[:, 1:2], -1.0,
                                        op0=mybir.AluOpType.mult,
                                        op1=mybir.AluOpType.mult)
                f = work.tile([128, D1], BF16, tag="f")
                nc.scalar.activation(f[:], y1b[:],
                                     mybir.ActivationFunctionType.Identity,
                                     bias=nmr[:], scale=mv[:, 1:2])
                nc.gpsimd.tensor_mul(fh_r[:, i, :], f[:], ln1g16[:])

        # ---- BatchNorm stats: read AR result, form A/Ball vectors ---------
        bnall = const.tile([1, 2 * D1], F32, tag="bnall")
        nc.scalar.dma_start(bnall[:], bn_out.ap()[:])

        mean_r = const.tile([1, D1], F32, tag="mean_r")
        nc.scalar.mul(mean_r[:], bnall[:, 0:D1], 1.0 / N)
        var_r = const.tile([1, D1], F32, tag="var_r")
        nc.scalar.mul(var_r[:], bnall[:, D1:2 * D1], 1.0 / N)
        msq = const.tile([1, D1], F32, tag="msq")
        nc.vector.tensor_mul(msq[:], mean_r[:], mean_r[:])
        nc.vector.tensor_sub(var_r[:], var_r[:], msq[:])
        nc.scalar.activation(var_r[:], var_r[:],
                             mybir.ActivationFunctionType.Sqrt, bias=eps1_sb[:])
        nc.vector.reciprocal(var_r[:], var_r[:])          # rstd
        ab_row = const.tile([1, 2 * D1], F32, tag="ab_row")
        A_row = ab_row[:, 0:D1]
        ball_row = ab_row[:, D1:2 * D1]
        nc.vector.tensor_mul(A_row, var_r[:], bng_row[:])
        mA = const.tile([1, D1], F32, tag="mA")
        nc.vector.tensor_mul(mA[:], mean_r[:], A_row)
        nc.vector.tensor_sub(ball_row, bnb_row[:], mA[:])
        nc.vector.tensor_add(ball_row, ball_row, ln1b_row[:])
        nc.scalar.dma_start(ab_d.ap()[0:1, :], ab_row[:])
        nc.scalar.dma_start(abt_t[:], ab_d.ap()[0:1, :].to_broadcast([128, 2 * D1]))
        abt16 = const.tile([128, 2 * D1], BF16, tag="abt16")
        nc.vector.tensor_copy(abt16[:], abt_t[:])
        A16 = abt16[:, 0:D1]
        Ball16 = abt16[:, D1:2 * D1]

        # ---- pass 2: h, hT, xw, scale, store (+chunked AG) ----------------
        with ExitStack() as p2:
            work = p2.enter_context(tc.tile_pool(name="work2", bufs=6))
            ps = p2.enter_context(tc.tile_pool(name="ps2", bufs=2, space="PSUM"))
            tps = p2.enter_context(tc.tile_pool(name="tps", bufs=3, space="PSUM"))

            q_cur = 0
            for i in range(NB):
                t = work.tile([128, D1], BF16, tag="t")
                nc.gpsimd.tensor_mul(t[:], ysc_r[:, i, :], A16)
                f = work.tile([128, D1], BF16, tag="f")
                nc.vector.tensor_add(f[:], t[:], fh_r[:, i, :])
                nc.vector.tensor_add(f[:], f[:], Ball16)
                h = work.tile([128, D1], BF16, tag="h")
                if USE_LRELU:
                    nc.scalar.activation(h[:], f[:],
                                         mybir.ActivationFunctionType.Lrelu,
                                         alpha=SLOPE)
                else:
                    hl = work.tile([128, D1], BF16, tag="hl")
                    nc.scalar.mul(hl[:], f[:], SLOPE)
                    nc.vector.tensor_max(h[:], f[:], hl[:])

                ht = work.tile([128, JC, 128], BF16, tag="ht")
                for j in range(JC):
                    tp = tps.tile([128, 128], BF16, tag="tp")
                    nc.tensor.transpose(tp[:], h[:, j * 128:(j + 1) * 128], ident_sb[:])
                    if j % 2 == 0:
                        nc.scalar.copy(ht[:, j, :], tp[:])
                    else:
                        nc.vector.tensor_copy(ht[:, j, :], tp[:])
                xwp = ps.tile([128, D2], F32, tag="xwp")
                for j in range(JC):
                    nc.tensor.matmul(xwp[:], ht[:, j, :], gcnwt_sb[:, j, :],
                                     start=(j == 0), stop=(j == JC - 1))
                xws = work.tile([128, D2], TDT, tag="xws")
                nc.vector.tensor_scalar(xws[:], xwp[:], dinv_sb[:, i:i + 1], None,
                                        op0=mybir.AluOpType.mult)
                q = q_cur
                nc.sync.dma_start(
                    xwsq[q].ap()[(i - qs[q]) * 128:(i - qs[q] + 1) * 128, :],
                    xws[:])
                if i + 1 == qs[q_cur + 1]:
                    # quarter complete: AllGather it into its table slice
                    nc.gpsimd.collective_compute(
                        "AllGather", mybir.AluOpType.bypass, replica_groups=rg,
                        ins=[xwsq[q].ap()[:]],
                        outs=[table.ap()[qbase[q]:qbase[q + 1], :]])
                    q_cur += 1

        p12.close()

        # ---- pass 3: gather + S-matmul aggregation + LN2 -----------------
        with ExitStack() as p3:
            mpool = p3.enter_context(tc.tile_pool(name="mpool", bufs=MSG_RING))
            spool = p3.enter_context(tc.tile_pool(name="spool", bufs=3))
            work = p3.enter_context(tc.tile_pool(name="work3", bufs=3))
            ps = p3.enter_context(tc.tile_pool(name="ps3", bufs=4, space="PSUM"))
            xwsr = p3.enter_context(tc.tile_pool(name="xwsr", bufs=1))

            # bulk-reload this core's scaled rows (self-loop term) from the
            # quarter scratch; one DMA per quarter, off the per-block path
            xws_r = xwsr.tile([128, NB, D2], TDT, tag="xws_r")
            for q in range(nq):
                nc.scalar.dma_start(
                    xws_r[:, qs[q]:qs[q + 1], :],
                    xwsq[q].ap().rearrange("(b p) d -> p b d", p=128))

            # call table: stream-packed gather calls; groups are 128-aligned
            # within calls, so each matmul group maps to one call slice.
            # stream "lo": groups [0, TL_lo/128); "hi": [TL_lo/128, TG)
            calls = []  # (stream, idx16_off, n_idx, first_group)
            for stream, tl, base_g, base_i in (
                    ("lo", TL_lo, 0, 0),
                    ("hi", TL_hi, TL_lo // 128, TL_lo // 16)):
                o = 0
                while o < tl:
                    n = min(SUB, tl - o)
                    calls.append((stream, base_i + o // 16, n, base_g + o // 128))
                    o += n
            n_lo_calls = sum(1 for c in calls if c[0] == "lo")

            call_tiles = {}
            emitted = {"lo": 0, "hi": 0}
            qrr = [0]
            tbl_lo = table.ap()[0:SPLIT, :]
            tbl_hi = table.ap()[SPLIT:TBL, :]

            def emit_call(ci):
                stream, ioff, n, g0 = calls[ci]
                mt = mpool.tile([128, GPC, D2], TDT, tag="msg")
                nc.gpsimd.dma_gather(
                    out_ap=mt[:, 0:n // 128, :],
                    in_ap=tbl_lo if stream == "lo" else tbl_hi,
                    idxs_ap=idx_all_sb[:, ioff:ioff + n // 16],
                    num_idxs=n, num_idxs_reg=reg_of[n], elem_size=D2,
                    single_packet=True, queue_num=qrr[0] % 4)
                qrr[0] += 1
                call_tiles[ci] = mt

            def group_view(g):
                """global group id -> [128, D2] subview of its call tile.

                Calls are emitted lazily per stream, in consumption order, so
                msg-ring WAR edges always point backwards (no cycles)."""
                if g < TL_lo // 128:
                    stream, base, ci_l = "lo", 0, g // GPC
                    goff = g - ci_l * GPC
                else:
                    gh = g - TL_lo // 128
                    stream, base, ci_l = "hi", n_lo_calls, gh // GPC
                    goff = gh - ci_l * GPC
                while emitted[stream] <= ci_l:
                    emit_call(base + emitted[stream])
                    emitted[stream] += 1
                return call_tiles[base + ci_l][:, goff, :]

            og = 0
            for b in range(NB):
                G = (L_lo[b] + L_hi[b]) // 128
                glist = []
                lo_g0 = sum(L_lo[:b]) // 128
                hi_g0 = TL_lo // 128 + sum(L_hi[:b]) // 128
                glist += [lo_g0 + k for k in range(L_lo[b] // 128)]
                glist += [hi_g0 + k for k in range(L_hi[b] // 128)]

                cols_sb = cols_all_sb[:, og:og + G]
                S_all = spool.tile([128, G, 128], TDT, tag="S_all")
                i3 = iotab_c[:, 0:G * 128].rearrange("p (g t) -> p g t", g=G)
                cb = cols_sb[:, 0:G].unsqueeze(2).broadcast_to((128, G, 128))
                if ISEQ_SWAP:
                    nc.vector.tensor_tensor(out=S_all[:], in0=cb, in1=i3,
                                            op=mybir.AluOpType.is_equal)
                else:
                    nc.vector.tensor_tensor(out=S_all[:], in0=i3, in1=cb,
                                            op=mybir.AluOpType.is_equal)

                acc = ps.tile([128, D2], F32, tag="acc")
                for k, g in enumerate(glist):
                    mv_view = group_view(g)
                    nc.tensor.matmul(acc[:], S_all[:, k, :], mv_view,
                                     start=(k == 0), stop=False)
                # self-loop: acc[t,:] += xws[t,:] via identity matmul
                if table_bf16:
                    nc.tensor.matmul(acc[:], ident_sb[:], xws_r[:, b, :],
                                     start=False, stop=True)
                else:
                    nc.tensor.matmul(acc[:], ident_sb[:].bitcast(F32R),
                                     xws_r[:, b, :], start=False, stop=True)

                ev = work.tile([128, D2], F32, tag="ev")
                nc.vector.tensor_scalar(ev[:], acc[:], dinv_sb[:, b:b + 1], None,
                                        op0=mybir.AluOpType.mult)
                nc.vector.tensor_add(ev[:], ev[:], gcnb_t[:])
                st = work.tile([128, 6], F32, tag="st3")
                nc.vector.bn_stats(st[:], ev[:])
                mv = work.tile([128, 2], F32, tag="mv3")
                nc.vector.bn_aggr(mv[:], st[:])
                nc.scalar.activation(mv[:, 1:2], mv[:, 1:2],
                                     mybir.ActivationFunctionType.Sqrt,
                                     bias=eps_sb[:])
                nc.vector.reciprocal(mv[:, 1:2], mv[:, 1:2])
                nmr = work.tile([128, 1], F32, tag="nmr3")
                nc.vector.tensor_scalar(nmr[:], mv[:, 0:1], mv[:, 1:2], -1.0,
                                        op0=mybir.AluOpType.mult,
                                        op1=mybir.AluOpType.mult)
                f2 = work.tile([128, D2], F32, tag="f2")
                nc.scalar.activation(f2[:], ev[:],
                                     mybir.ActivationFunctionType.Identity,
                                     bias=nmr[:], scale=mv[:, 1:2])
                nc.vector.tensor_mul(f2[:], f2[:], ln2g_t[:])
                nc.vector.tensor_add(f2[:], f2[:], ln2b_t[:])
                # leaky via mul+max: Lrelu here would thrash the act table
                # against Sqrt/Identity every block
                oo = work.tile([128, D2], F32, tag="oo")
                ol = work.tile([128, D2], F32, tag="ol")
                nc.scalar.mul(ol[:], f2[:], SLOPE)
                nc.vector.tensor_max(oo[:], f2[:], ol[:])
                nc.sync.dma_start(out_d[b * 128:(b + 1) * 128, :], oo[:])
                og += G

    nc.compile()
    return nc


_last_results = None


def kernel(**inputs) -> np.ndarray:
    global _last_results
    in_maps, L_lo, L_hi, cfg = preprocess(**inputs)
    table_bf16 = os.environ.get("GNN_TABLE_FP32", "") != "1"
    nc = build_program(cfg, L_lo, L_hi, table_bf16=table_bf16)
    trace = os.environ.get("GNN_TRACE", "") == "1"
    res = run_bass_kernel_spmd(nc, in_maps, core_ids=list(range(cfg["P"])),
                               trace=trace)
    _last_results = res
    NC = cfg["NC"]
    return np.concatenate([res.results[c]["out"][:NC] for c in range(cfg["P"])],
                          axis=0)


# revision 18
# speedup vs baseline: 1.1463x; 1.1135x over previous
"""Trainium2 Bass kernel for nn_ATAC_Encoder (GCN message passing), 8 cores.

Math (reference):
    f  = LayerNorm(x @ W1.T + b1) * ln1_g + ln1_b
    sc = BatchNorm(x @ sc_W.T + sc_b) * bn_g + bn_b      (batch stats over nodes)
    h  = leaky_relu(f + sc, 0.01)
    g  = GCNConv(h, edge_index, gcn_W, gcn_b)            (sym-norm, self-loops)
    out = leaky_relu(LayerNorm(g) * ln2_g + ln2_b, 0.01)

Distribution: nodes are block-sharded across 8 NeuronCores (6250 each, padded
to 6272 = 49*128). Each core computes its own h and xw = h @ gcn_W.T, scales
rows by dinv (deg^-1/2), AllGathers the scaled table (in 4 node-quarter
chunks so the collective overlaps the tail of the dense pass), then gathers
per-edge source rows (edges bucketed on host by target core / target
128-block) with single-packet SWDGE dma_gather calls round-robined over 4
SWDGE queues, and aggregates them with selector-matrix matmuls on the tensor
engine:
    acc[t, :] += S_g.T @ msgs_g     with S_g[p, t] = (col[p] == t)
The S matrices for a whole block are built in ONE vector-engine is_equal op
(iota row compared against a stride-0-broadcast column tensor). BatchNorm
statistics use a ones-vector matmul partial sum + a tiny AllReduce. The sc_W
bias cancels in BatchNorm and is dropped; GCN self-loops are appended as
ordinary edges. dma_gather indices are int16, so gathers are split into a
low-table / high-table call pair against offset views.

float32r (20-bit fp32, 1 cycle/row on the PE) is used for the dense matmuls.
The gathered message table is bf16 by default (halves gather traffic;
~1e-3 relative error) — set GNN_TABLE_FP32=1 for an fp32r table.
"""

import os
import numpy as np
from contextlib import ExitStack

import concourse.bass as bass
import concourse.mybir as mybir
import concourse.tile as tile
from concourse import bacc
from concourse.bass_utils import run_bass_kernel_spmd

F32 = mybir.dt.float32
F32R = mybir.dt.float32r
BF16 = mybir.dt.bfloat16
F16 = mybir.dt.float16
I16 = mybir.dt.int16

LN_EPS = 1e-5
BN_EPS = 1e-5
SLOPE = 0.01
GATHER_SUB = 896          # max indices per single-packet dma_gather call
NQ = 4                    # AllGather chunks (node quarters)


def _round_f32r(a: np.ndarray) -> np.ndarray:
    """Round fp32 to float32r (11-bit mantissa; low 12 bits zero)."""
    u = a.astype(np.float32).view(np.uint32)
    u = (u + 0x800) & np.uint32(0xFFFFF000)
    return u.view(np.float32)


def _wrap16(v: np.ndarray, L: int) -> np.ndarray:
    """[L] index list -> [128, L/16] int16 (16-lane wrap, replicated 8x)."""
    a = np.zeros(L, np.int16)
    a[: len(v)] = v
    return np.tile(a.reshape(L // 16, 16).T, (8, 1))


def _cols_wrap(v: np.ndarray, L: int) -> np.ndarray:
    """[cnt] col list -> [128, L/128] fp32 (slot m -> [m%128, m//128]), pad -1."""
    a = np.full(L, -1.0, np.float32)
    a[: len(v)] = v
    return a.reshape(L // 128, 128).T


def preprocess(x, edge_index, W1, b1, ln1_g, ln1_b, sc_W, sc_b, bn_g, bn_b,
               gcn_W, gcn_b, ln2_g, ln2_b, n_cores=8, split=32768):
    """Shard inputs; returns (in_maps, L_lo, L_hi, cfg)."""
    x = np.asarray(x, np.float32)
    ei = np.asarray(edge_index)
    N, D0 = x.shape
    D1 = W1.shape[0]
    D2 = gcn_W.shape[0]
    P = n_cores
    NC = N // P
    NPAD = ((NC + 127) // 128) * 128
    NB = NPAD // 128
    TBL = P * NPAD

    # node-quarter chunking of the AllGather: quarter q covers blocks
    # [qs[q], qs[q+1]); table rows are laid out quarter-major then core-major.
    nq = min(NQ, NB)
    # front-load quarters so the last AllGather (which gates the first
    # gather) moves the least data
    if nq == NQ and NB > 2 * nq:
        last = 2
        big = (NB - last) // (nq - 1)
        szs = [big] * (nq - 1) + [NB - last - big * (nq - 2)]
        szs[-1] = NB - sum(szs[:-1])
        qs = [0]
        for z in szs:
            qs.append(qs[-1] + z)
    else:
        qs = [round(q * NB / nq) for q in range(nq + 1)]
    qsz = np.array([qs[q + 1] - qs[q] for q in range(nq)])
    qbase = np.cumsum([0] + [P * int(s) * 128 for s in qsz])
    q_of_block = np.repeat(np.arange(nq), qsz)
    qs_arr = np.array(qs[:nq])

    # self-loops appended as ordinary edges
    loops = np.arange(N, dtype=np.int64)
    row = np.concatenate([ei[0].astype(np.int64), loops])
    col = np.concatenate([ei[1].astype(np.int64), loops])

    deg = np.bincount(col, minlength=N).astype(np.float64)
    dinv = (1.0 / np.sqrt(deg)).astype(np.float32)  # deg >= 1 (self-loops)

    sown = row // NC
    sloc = row - sown * NC
    sblk = sloc >> 7
    sq = q_of_block[sblk]
    trow = qbase[sq] + sown * (qsz[sq] * 128) + (sloc - qs_arr[sq] * 128)
    owner = col // NC
    lcol = col - owner * NC

    # bucket edges per (core, target block, lo/hi table half)
    per = []
    for c in range(P):
        m = owner == c
        tr, lc = trow[m], lcol[m]
        blk = lc >> 7
        hi = (tr >= split).astype(np.int64)
        order = np.lexsort((tr, hi, blk))
        tr, lc, blk, hi = tr[order], lc[order], blk[order], hi[order]
        bounds = np.searchsorted(blk * 2 + hi, np.arange(2 * NB + 1))
        blocks = []
        for b in range(NB):
            lo_s, lo_e = bounds[2 * b], bounds[2 * b + 1]
            hi_s, hi_e = bounds[2 * b + 1], bounds[2 * b + 2]
            blocks.append((tr[lo_s:lo_e], tr[hi_s:hi_e] - split,
                           lc[lo_s:lo_e] - b * 128, lc[hi_s:hi_e] - b * 128))
        per.append(blocks)

    def rup128(n):
        return ((n + 127) // 128) * 128

    L_lo = [rup128(max(len(per[c][b][0]) for c in range(P))) for b in range(NB)]
    L_hi = [rup128(max(len(per[c][b][1]) for c in range(P))) for b in range(NB)]

    TI = sum(L_lo[b] + L_hi[b] for b in range(NB)) // 16
    TG = sum(L_lo[b] + L_hi[b] for b in range(NB)) // 128
    Gmax = max((L_lo[b] + L_hi[b]) // 128 for b in range(NB))

    from ml_dtypes import bfloat16
    

    KC0 = D0 // 128
    w1t = np.ascontiguousarray(W1.T).astype(bfloat16)      # [D0, D1]
    scwt = np.ascontiguousarray(sc_W.T).astype(bfloat16)   # [D0, D1]
    gcnwt = np.ascontiguousarray(gcn_W.T).astype(bfloat16)  # [D1, D2]
    iota = np.broadcast_to(np.arange(128, dtype=np.float32), (128, 128))
    iota_big = np.ascontiguousarray(np.tile(iota, (1, Gmax))).astype(bfloat16)
    ident = np.eye(128, dtype=np.float32).astype(bfloat16)
    ones = np.ones((128, 1), np.float32)

    vec = lambda v: np.asarray(v, np.float32).reshape(1, -1)

    in_maps = []
    for c in range(P):
        xp = np.zeros((D0, NPAD), np.float32)
        xp[:, :NC] = x[c * NC:(c + 1) * NC].T
        # pretile: [NB, p(d0 in k-chunk), k, n] so each chunk is one
        # contiguous [128, KC*128] DMA
        xp = np.ascontiguousarray(
            xp.reshape(KC0, 128, NB, 128).transpose(2, 1, 0, 3)).astype(bfloat16)

        dv = np.zeros(NPAD, np.float32)
        dv[:NC] = dinv[c * NC:(c + 1) * NC]
        dvT = np.ascontiguousarray(dv.reshape(NB, 128).T)  # [128, NB]

        idx_all = np.zeros((128, TI), np.int16)
        cols_all = np.full((128, TG), -1.0, np.float32)
        counts = []

        def seg_idx(vals, L):
            a = np.full(L, -1, np.int16)
            a[: len(vals)] = vals
            o = 0
            while o < L:
                n = min(GATHER_SUB, L - o)
                cnt = min(max(len(vals) - o, 16), n)
                a[max(o, len(vals)): o + cnt] = 0   # padded-but-gathered
                counts.append(cnt)
                o += n
            return a

        oi = og = 0
        for b in range(NB):
            lo_t, hi_t, lo_c, hi_c = per[c][b]
            ll, lh = L_lo[b], L_hi[b]
            av = seg_idx(lo_t.astype(np.int16), ll)
            idx_all[:, oi:oi + ll // 16] = np.tile(av.reshape(ll // 16, 16).T, (8, 1))
            if lh:
                bv = seg_idx(hi_t.astype(np.int16), lh)
                idx_all[:, oi + ll // 16: oi + (ll + lh) // 16] = np.tile(
                    bv.reshape(lh // 16, 16).T, (8, 1))
            cols_all[:, og:og + ll // 128] = _cols_wrap(lo_c.astype(np.float32), ll)
            cols_all[:, og + ll // 128: og + (ll + lh) // 128] = _cols_wrap(hi_c.astype(np.float32), lh)
            oi += (ll + lh) // 16
            og += (ll + lh) // 128
        gcnt = np.asarray(counts, np.int32).reshape(1, -1)

        cols_all = cols_all.astype(bfloat16)

        in_maps.append({
            "xt": xp, "w1t": w1t, "scwt": scwt, "gcnwt": gcnwt,
            "b1": vec(b1), "ln1_g": vec(ln1_g), "ln1_b": vec(ln1_b),
            "bn_g": vec(bn_g), "bn_b": vec(bn_b),
            "gcn_b": vec(gcn_b), "ln2_g": vec(ln2_g), "ln2_b": vec(ln2_b),
            "idx": idx_all, "cols": cols_all, "dinvT": dvT, "gcnt": gcnt,
            "iota_big": iota_big, "ident": ident, "ones": ones,
        })

    cfg = dict(P=P, N=N, NC=NC, NPAD=NPAD, NB=NB, D0=D0, D1=D1, D2=D2,
               NCALLS=len(counts),
               TBL=TBL, SPLIT=split, TI=TI, TG=TG, Gmax=Gmax,
               qs=[int(v) for v in qs], qsz=[int(v) for v in qsz],
               qbase=[int(v) for v in qbase])
    return in_maps, L_lo, L_hi, cfg


def build_program(cfg, L_lo, L_hi, table_bf16=True):
    P, N = cfg["P"], cfg["N"]
    NPAD, NB = cfg["NPAD"], cfg["NB"]
    D0, D1, D2 = cfg["D0"], cfg["D1"], cfg["D2"]
    TBL, SPLIT, Gmax = cfg["TBL"], cfg["SPLIT"], cfg["Gmax"]
    qs, qsz, qbase = cfg["qs"], cfg["qsz"], cfg["qbase"]
    nq = len(qsz)
    KC, JC = D0 // 128, D1 // 128
    rg = [list(range(P))]
    TDT = BF16 if table_bf16 else F32R

    nc = bacc.Bacc("TRN2", target_bir_lowering=False, debug=False, num_devices=P,
                   num_swdge_queues=4)

    xt_d = nc.dram_tensor("xt", [NB, 128, KC, 128], BF16, kind="ExternalInput").ap()
    w1t_d = nc.dram_tensor("w1t", [D0, D1], BF16, kind="ExternalInput").ap()
    scwt_d = nc.dram_tensor("scwt", [D0, D1], BF16, kind="ExternalInput").ap()
    gcnwt_d = nc.dram_tensor("gcnwt", [D1, D2], BF16, kind="ExternalInput").ap()
    b1_d = nc.dram_tensor("b1", [1, D1], F32, kind="ExternalInput").ap()
    ln1g_d = nc.dram_tensor("ln1_g", [1, D1], F32, kind="ExternalInput").ap()
    ln1b_d = nc.dram_tensor("ln1_b", [1, D1], F32, kind="ExternalInput").ap()
    bng_d = nc.dram_tensor("bn_g", [1, D1], F32, kind="ExternalInput").ap()
    bnb_d = nc.dram_tensor("bn_b", [1, D1], F32, kind="ExternalInput").ap()
    gcnb_d = nc.dram_tensor("gcn_b", [1, D2], F32, kind="ExternalInput").ap()
    ln2g_d = nc.dram_tensor("ln2_g", [1, D2], F32, kind="ExternalInput").ap()
    ln2b_d = nc.dram_tensor("ln2_b", [1, D2], F32, kind="ExternalInput").ap()
    idx_d = nc.dram_tensor("idx", [128, cfg["TI"]], I16, kind="ExternalInput").ap()
    cols_d = nc.dram_tensor("cols", [128, cfg["TG"]], BF16, kind="ExternalInput").ap()
    dinv_d = nc.dram_tensor("dinvT", [128, NB], F32, kind="ExternalInput").ap()
    iotab_d = nc.dram_tensor("iota_big", [128, Gmax * 128], BF16, kind="ExternalInput").ap()
    ident_d = nc.dram_tensor("ident", [128, 128], BF16, kind="ExternalInput").ap()
    ones_d = nc.dram_tensor("ones", [128, 1], F32, kind="ExternalInput").ap()
    gcnt_d = nc.dram_tensor("gcnt", [1, cfg["NCALLS"]], mybir.dt.int32,
                            kind="ExternalInput").ap()
    out_d = nc.dram_tensor("out", [NPAD, D2], F32, kind="ExternalOutput").ap()

    xwsq = [nc.dram_tensor(f"xwsq{q}", [qsz[q] * 128, D2], TDT) for q in range(nq)]
    table = nc.dram_tensor("table", [TBL, D2], TDT, addr_space="Shared")
    bn_in = nc.dram_tensor("bn_in", [1, 2 * D1], F32)
    bn_out = nc.dram_tensor("bn_out", [1, 2 * D1], F32, addr_space="Shared")
    ab_d = nc.dram_tensor("ab_d", [1, 2 * D1], F32)      # A|Ball bounce

    r = lambda ap: ap.bitcast(F32R)

    with tile.TileContext(nc) as tc, ExitStack() as ctx:
        const = ctx.enter_context(tc.tile_pool(name="const", bufs=1))

        def const_load(name, dram_ap, shape, dt=F32, bcast=False):
            t = const.tile(shape, dt, tag=name)
            src = dram_ap.to_broadcast(shape) if bcast else dram_ap
            if dt == F32R:
                src = src.bitcast(F32R)
            nc.sync.dma_start(t[:], src)
            return t

        w1t_sb = const.tile([128, KC, D1], BF16, tag="w1t_sb")
        nc.sync.dma_start(w1t_sb[:], w1t_d.rearrange("(k p) n -> p k n", p=128))
        scwt_sb = const.tile([128, KC, D1], BF16, tag="scwt_sb")
        nc.sync.dma_start(scwt_sb[:], scwt_d.rearrange("(k p) n -> p k n", p=128))
        gcnwt_sb = const.tile([128, JC, D2], BF16, tag="gcnwt_sb")
        nc.sync.dma_start(gcnwt_sb[:], gcnwt_d.rearrange("(k p) n -> p k n", p=128))

        b1_t = const_load("b1_t", b1_d, [128, D1], bcast=True)
        ln1g_t = const_load("ln1g_t", ln1g_d, [128, D1], bcast=True)
        gcnb_t = const_load("gcnb_t", gcnb_d, [128, D2], bcast=True)
        ln2g_t = const_load("ln2g_t", ln2g_d, [128, D2], bcast=True)
        ln2b_t = const_load("ln2b_t", ln2b_d, [128, D2], bcast=True)
        iotab_c = const_load("iotab_c", iotab_d, [128, Gmax * 128], dt=BF16)
        ident_sb = const_load("ident_sb", ident_d, [128, 128], dt=BF16)
        ones_sb = const_load("ones_sb", ones_d, [128, 1])
        dinv_sb = const_load("dinv_sb", dinv_d, [128, NB])
        ln1b_row = const_load("ln1b_row", ln1b_d, [1, D1])
        bng_row = const_load("bng_row", bng_d, [1, D1])
        bnb_row = const_load("bnb_row", bnb_d, [1, D1])
        idx_all_sb = const.tile([128, cfg["TI"]], I16, tag="idx_all_sb")
        nc.sync.dma_start(idx_all_sb[:], idx_d[:])
        gcnt_sb = const.tile([1, cfg["NCALLS"]], mybir.dt.int32, tag="gcnt_sb")
        nc.sync.dma_start(gcnt_sb[:], gcnt_d[:])
        cols_all_sb = const.tile([128, cfg["TG"]], BF16, tag="cols_all_sb")
        nc.sync.dma_start(cols_all_sb[:], cols_d[:])

        eps_sb = const.tile([128, 1], F32, tag="eps_sb")
        nc.vector.memset(eps_sb[:], LN_EPS)
        eps1_sb = const.tile([1, 1], F32, tag="eps1_sb")
        nc.vector.memset(eps1_sb[:], BN_EPS)

        abt_t = const.tile([128, 2 * D1], F32, tag="abt_t")
        A_t = abt_t[:, 0:D1]
        Ball_t = abt_t[:, D1:2 * D1]

        xt_v = xt_d  # [NB, 128, KC, 128] chunk-contiguous, bf16

        p12 = ExitStack()
        resid = p12.enter_context(tc.tile_pool(name="resid", bufs=1))
        fh_r = resid.tile([128, NB, D1], BF16, tag="fh_r", name="fh_r")
        ysc_r = resid.tile([128, NB, D1], BF16, tag="ysc_r", name="ysc_r")

        # ---- pass 1a: ysc = x @ scW.T ; BN sums via vector accumulation --
        # (the tiny BN AllReduce is kicked right after, so it overlaps the
        # y1 GEMM of pass 1b instead of stalling the whole device)
        bn_sb = const.tile([1, 2 * D1], F32, tag="bn_sb")
        S_acc = const.tile([128, D1], F32, tag="S_acc")
        Q_acc = const.tile([128, D1], F32, tag="Q_acc")
        nc.vector.memset(S_acc[:], 0.0)
        nc.vector.memset(Q_acc[:], 0.0)
        with ExitStack() as p1:
            xpool = p1.enter_context(tc.tile_pool(name="xpool", bufs=3))
            work = p1.enter_context(tc.tile_pool(name="work1", bufs=3))
            ps = p1.enter_context(tc.tile_pool(name="ps1", bufs=2, space="PSUM"))

            for i in range(NB):
                xt = xpool.tile([128, KC, 128], BF16, tag="xt")
                nc.sync.dma_start(xt[:], xt_v[i])
                yscp = ps.tile([128, D1], F32, tag="yscp")
                for k in range(KC):
                    nc.tensor.matmul(yscp[:], xt[:, k, :], scwt_sb[:, k, :],
                                     start=(k == 0), stop=(k == KC - 1))
                ysc = ysc_r[:, i, :]
                nc.scalar.copy(ysc, yscp[:])
                sq = work.tile([128, D1], BF16, tag="sq")
                nc.gpsimd.tensor_mul(sq[:], ysc, ysc)
                nc.vector.tensor_add(S_acc[:], S_acc[:], yscp[:])
                nc.vector.tensor_add(Q_acc[:], Q_acc[:], sq[:])

            sum_ps = p1.enter_context(tc.tile_pool(name="sum_ps", bufs=1, space="PSUM"))
            sump = sum_ps.tile([1, D1], F32, tag="sump")
            sqsump = sum_ps.tile([1, D1], F32, tag="sqsump")
            nc.tensor.matmul(sump[:], ones_sb[:], S_acc[:], start=True, stop=True)
            nc.tensor.matmul(sqsump[:], ones_sb[:], Q_acc[:], start=True, stop=True)
            nc.vector.tensor_copy(bn_sb[:, 0:D1], sump[:])
            nc.vector.tensor_copy(bn_sb[:, D1:2 * D1], sqsump[:])

        nc.scalar.dma_start(bn_in.ap()[:], bn_sb[:])
        nc.gpsimd.collective_compute(
            "AllReduce", mybir.AluOpType.add, replica_groups=rg,
            ins=[bn_in.ap()[:]], outs=[bn_out.ap()[:]])

        # ---- pass 1b: y1 = x @ W1.T ; LayerNorm1 (overlaps the AllReduce)
        with ExitStack() as p1b:
            xpool = p1b.enter_context(tc.tile_pool(name="xpool1b", bufs=3))
            work = p1b.enter_context(tc.tile_pool(name="work1b", bufs=3))
            ps = p1b.enter_context(tc.tile_pool(name="ps1b", bufs=2, space="PSUM"))

            for i in range(NB):
                xt = xpool.tile([128, KC, 128], BF16, tag="xt")
                nc.sync.dma_start(xt[:], xt_v[i])
                y1p = ps.tile([128, D1], F32, tag="y1p")
                for k in range(KC):
                    nc.tensor.matmul(y1p[:], xt[:, k, :], w1t_sb[:, k, :],
                                     start=(k == 0), stop=(k == KC - 1))
                # LayerNorm1 (per-node, no cross-core stats needed)
                y1b = work.tile([128, D1], F32, tag="y1b")
                nc.vector.tensor_add(y1b[:], y1p[:], b1_t[:])
                st = work.tile([128, 6], F32, tag="st")
                nc.vector.bn_stats(st[:], y1b[:])
                mv = work.tile([128, 2], F32, tag="mv")
                nc.vector.bn_aggr(mv[:], st[:])
                nc.scalar.activation(mv[:, 1:2], mv[:, 1:2],
                                     mybir.ActivationFunctionType.Sqrt,
                                     bias=eps_sb[:])
                nc.vector.reciprocal(mv[:, 1:2], mv[:, 1:2])
                nmr = work.tile([128, 1], F32, tag="nmr")
                nc.vector.tensor_scalar(nmr[:], mv[:, 0:1], mv[:, 1:2], -1.0,
                                        op0=mybir.AluOpType.mult,
                                        op1=mybir.AluOpType.mult)
                f = work.tile([128, D1], F32, tag="f")
                nc.scalar.activation(f[:], y1b[:],
                                     mybir.ActivationFunctionType.Identity,
                                     bias=nmr[:], scale=mv[:, 1:2])
                nc.gpsimd.tensor_mul(fh_r[:, i, :], f[:], ln1g_t[:])

        # ---- BatchNorm stats: read AR result, form A/Ball vectors --------
        bnall = const.tile([1, 2 * D1], F32, tag="bnall")
        nc.scalar.dma_start(bnall[:], bn_out.ap()[:])

        mean_r = const.tile([1, D1], F32, tag="mean_r")
        nc.scalar.mul(mean_r[:], bnall[:, 0:D1], 1.0 / N)
        var_r = const.tile([1, D1], F32, tag="var_r")
        nc.scalar.mul(var_r[:], bnall[:, D1:2 * D1], 1.0 / N)
        msq = const.tile([1, D1], F32, tag="msq")
        nc.vector.tensor_mul(msq[:], mean_r[:], mean_r[:])
        nc.vector.tensor_sub(var_r[:], var_r[:], msq[:])
        nc.scalar.activation(var_r[:], var_r[:],
                             mybir.ActivationFunctionType.Sqrt, bias=eps1_sb[:])
        nc.vector.reciprocal(var_r[:], var_r[:])          # rstd
        ab_row = const.tile([1, 2 * D1], F32, tag="ab_row")
        A_row = ab_row[:, 0:D1]
        ball_row = ab_row[:, D1:2 * D1]
        nc.vector.tensor_mul(A_row, var_r[:], bng_row[:])
        mA = const.tile([1, D1], F32, tag="mA")
        nc.vector.tensor_mul(mA[:], mean_r[:], A_row)
        nc.vector.tensor_sub(ball_row, bnb_row[:], mA[:])
        nc.vector.tensor_add(ball_row, ball_row, ln1b_row[:])
        nc.scalar.dma_start(ab_d.ap()[0:1, :], ab_row[:])
        nc.scalar.dma_start(abt_t[:], ab_d.ap()[0:1, :].to_broadcast([128, 2 * D1]))
        abt16 = const.tile([128, 2 * D1], BF16, tag="abt16")
        nc.vector.tensor_copy(abt16[:], abt_t[:])
        A16 = abt16[:, 0:D1]
        Ball16 = abt16[:, D1:2 * D1]

        # ---- pass 2: y1, LN1, h, hT, xw, scale, store (+chunked AG) ------
        with ExitStack() as p2:
            xpool = p2.enter_context(tc.tile_pool(name="xpool2", bufs=3))
            work = p2.enter_context(tc.tile_pool(name="work2", bufs=3))
            ps = p2.enter_context(tc.tile_pool(name="ps2", bufs=2, space="PSUM"))
            tps = p2.enter_context(tc.tile_pool(name="tps", bufs=3, space="PSUM"))

            q_cur = 0
            for i in range(NB):
                t = work.tile([128, D1], BF16, tag="t")
                nc.gpsimd.tensor_mul(t[:], ysc_r[:, i, :], A16)
                f = work.tile([128, D1], BF16, tag="f")
                nc.vector.tensor_add(f[:], t[:], fh_r[:, i, :])
                nc.vector.tensor_add(f[:], f[:], Ball16)
                hl = work.tile([128, D1], BF16, tag="hl")
                nc.scalar.mul(hl[:], f[:], SLOPE)
                h = work.tile([128, D1], BF16, tag="h")
                nc.vector.tensor_max(h[:], f[:], hl[:])

                ht = work.tile([128, JC, 128], BF16, tag="ht")
                for j in range(JC):
                    tp = tps.tile([128, 128], BF16, tag="tp")
                    nc.tensor.transpose(tp[:], h[:, j * 128:(j + 1) * 128], ident_sb[:])
                    nc.scalar.copy(ht[:, j, :], tp[:])
                xwp = ps.tile([128, D2], F32, tag="xwp")
                for j in range(JC):
                    nc.tensor.matmul(xwp[:], ht[:, j, :], gcnwt_sb[:, j, :],
                                     start=(j == 0), stop=(j == JC - 1))
                xws = work.tile([128, D2], TDT, tag="xws")
                nc.vector.tensor_scalar(xws[:], xwp[:], dinv_sb[:, i:i + 1], None,
                                        op0=mybir.AluOpType.mult)
                q = q_cur
                nc.sync.dma_start(
                    xwsq[q].ap()[(i - qs[q]) * 128:(i - qs[q] + 1) * 128, :],
                    xws[:])
                if i + 1 == qs[q_cur + 1]:
                    # quarter complete: AllGather it into its table slice
                    nc.gpsimd.collective_compute(
                        "AllGather", mybir.AluOpType.bypass, replica_groups=rg,
                        ins=[xwsq[q].ap()[:]],
                        outs=[table.ap()[qbase[q]:qbase[q + 1], :]])
                    q_cur += 1

        p12.close()

        # ---- pass 3: gather + S-matmul aggregation + LN2 -----------------
        with ExitStack() as p3:
            nb3 = 3 if table_bf16 else 2
            mpool = p3.enter_context(tc.tile_pool(name="mpool", bufs=1))
            spool = p3.enter_context(tc.tile_pool(name="spool", bufs=nb3))
            work = p3.enter_context(tc.tile_pool(name="work3", bufs=3))
            ps = p3.enter_context(tc.tile_pool(name="ps3", bufs=4, space="PSUM"))

            # persistent msg ring, zeroed once: dynamic-count gathers leave
            # tail slots unwritten, and PSUM matmuls must never see NaN
            msg_ring = [mpool.tile([128, Gmax, D2], TDT, tag=f"msgr{r}", name=f"msgr{r}")
                        for r in range(6)]
            for mt in msg_ring:
                nc.vector.memset(mt[:], 0.0)

            qc = 0  # SWDGE queue round-robin
            oi = og = 0
            for b in range(NB):
                ll, lh = L_lo[b], L_hi[b]
                G = (ll + lh) // 128
                idx_sb = idx_all_sb[:, oi:oi + (ll + lh) // 16]
                cols_sb = cols_all_sb[:, og:og + G]

                msg = msg_ring[b % 6][:, 0:G, :]
                for base, ln in ((0, ll), (ll, lh)):
                    if ln == 0:
                        continue
                    tbl = table.ap()[0:SPLIT, :] if base == 0 else table.ap()[SPLIT:TBL, :]
                    o = 0
                    while o < ln:
                        n = min(GATHER_SUB, ln - o)
                        with nc.gpsimd.register() as creg:
                            nc.gpsimd.load(creg, gcnt_sb[0:1, qc:qc + 1])
                            nc.gpsimd.dma_gather(
                                out_ap=msg[:, (base + o) // 128:(base + o + n) // 128, :],
                                in_ap=tbl,
                                idxs_ap=idx_sb[:, (base + o) // 16:(base + o + n) // 16],
                                num_idxs=n, num_idxs_reg=creg, elem_size=D2,
                                single_packet=True, queue_num=qc % 4)
                        qc += 1
                        o += n

                S_all = spool.tile([128, G, 128], TDT, tag="S_all")
                nc.vector.tensor_tensor(
                    out=S_all[:],
                    in0=iotab_c[:, 0:G * 128].rearrange("p (g t) -> p g t", g=G),
                    in1=cols_sb[:, 0:G].unsqueeze(2).broadcast_to((128, G, 128)),
                    op=mybir.AluOpType.is_equal)

                acc = ps.tile([128, D2], F32, tag="acc")
                for g in range(G):
                    nc.tensor.matmul(acc[:], S_all[:, g, :], msg[:, g, :],
                                     start=(g == 0), stop=(g == G - 1))

                ev = work.tile([128, D2], F32, tag="ev")
                nc.vector.tensor_scalar(ev[:], acc[:], dinv_sb[:, b:b + 1], None,
                                        op0=mybir.AluOpType.mult)
                nc.vector.tensor_add(ev[:], ev[:], gcnb_t[:])
                st = work.tile([128, 6], F32, tag="st3")
                nc.vector.bn_stats(st[:], ev[:])
                mv = work.tile([128, 2], F32, tag="mv3")
                nc.vector.bn_aggr(mv[:], st[:])
                nc.scalar.activation(mv[:, 1:2], mv[:, 1:2],
                                     mybir.ActivationFunctionType.Sqrt,
                                     bias=eps_sb[:])
                nc.vector.reciprocal(mv[:, 1:2], mv[:, 1:2])
                nmr = work.tile([128, 1], F32, tag="nmr3")
                nc.vector.tensor_scalar(nmr[:], mv[:, 0:1], mv[:, 1:2], -1.0,
                                        op0=mybir.AluOpType.mult,
                                        op1=mybir.AluOpType.mult)
                f2 = work.tile([128, D2], F32, tag="f2")
                nc.scalar.activation(f2[:], ev[:],
                                     mybir.ActivationFunctionType.Identity,
                                     bias=nmr[:], scale=mv[:, 1:2])
                nc.vector.tensor_mul(f2[:], f2[:], ln2g_t[:])
                nc.vector.tensor_add(f2[:], f2[:], ln2b_t[:])
                ol = work.tile([128, D2], F32, tag="ol")
                nc.scalar.mul(ol[:], f2[:], SLOPE)
                oo = work.tile([128, D2], F32, tag="oo")
                nc.vector.tensor_max(oo[:], f2[:], ol[:])
                nc.sync.dma_start(out_d[b * 128:(b + 1) * 128, :], oo[:])

                oi += (ll + lh) // 16
                og += G

    nc.compile()
    return nc


_last_results = None


def kernel(**inputs) -> np.ndarray:
    global _last_results
    in_maps, L_lo, L_hi, cfg = preprocess(**inputs)
    table_bf16 = os.environ.get("GNN_TABLE_FP32", "") != "1"
    nc = build_program(cfg, L_lo, L_hi, table_bf16=table_bf16)
    trace = os.environ.get("GNN_TRACE", "") == "1"
    res = run_bass_kernel_spmd(nc, in_maps, core_ids=list(range(cfg["P"])),
                               trace=trace)
    _last_results = res
    NC = cfg["NC"]
    return np.concatenate([res.results[c]["out"][:NC] for c in range(cfg["P"])],
                          axis=0)



# revision 26
# speedup vs baseline: 1.1910x; 1.0390x over previous
"""Trainium2 Bass kernel for nn_ATAC_Encoder (GCN message passing), 8 cores.

Math (reference):
    f  = LayerNorm(x @ W1.T + b1) * ln1_g + ln1_b
    sc = BatchNorm(x @ sc_W.T + sc_b) * bn_g + bn_b      (batch stats over nodes)
    h  = leaky_relu(f + sc, 0.01)
    g  = GCNConv(h, edge_index, gcn_W, gcn_b)            (sym-norm, self-loops)
    out = leaky_relu(LayerNorm(g) * ln2_g + ln2_b, 0.01)

Distribution: nodes are block-sharded across 8 NeuronCores (6250 each, padded
to 6272 = 49*128). Each core computes its own h and xw = h @ gcn_W.T, scales
rows by dinv (deg^-1/2), AllGathers the scaled table (in 4 node-quarter
chunks so the collective overlaps the tail of the dense pass), then gathers
per-edge source rows (edges bucketed on host by target core / target
128-block) with single-packet SWDGE dma_gather calls round-robined over 4
SWDGE queues, and aggregates them with selector-matrix matmuls on the tensor
engine:
    acc[t, :] += S_g.T @ msgs_g     with S_g[p, t] = (col[p] == t)
The S matrices for a whole block are built in ONE vector-engine is_equal op
(iota row compared against a stride-0-broadcast column tensor). BatchNorm
statistics use a ones-vector matmul partial sum + a tiny AllReduce. The sc_W
bias cancels in BatchNorm and is dropped; GCN self-loops are appended as
ordinary edges. dma_gather indices are int16, so gathers are split into a
low-table / high-table call pair against offset views.

float32r (20-bit fp32, 1 cycle/row on the PE) is used for the dense matmuls.
The gathered message table is bf16 by default (halves gather traffic;
~1e-3 relative error) — set GNN_TABLE_FP32=1 for an fp32r table.
"""

import os
import numpy as np
from contextlib import ExitStack

import concourse.bass as bass
import concourse.mybir as mybir
import concourse.tile as tile
from concourse import bacc
from concourse.bass_utils import run_bass_kernel_spmd

F32 = mybir.dt.float32
F32R = mybir.dt.float32r
BF16 = mybir.dt.bfloat16
F16 = mybir.dt.float16
I16 = mybir.dt.int16

LN_EPS = 1e-5
BN_EPS = 1e-5
SLOPE = 0.01
GATHER_SUB = 896          # max indices per single-packet dma_gather call
NQ = 4                    # AllGather chunks (node quarters)


def _round_f32r(a: np.ndarray) -> np.ndarray:
    """Round fp32 to float32r (11-bit mantissa; low 12 bits zero)."""
    u = a.astype(np.float32).view(np.uint32)
    u = (u + 0x800) & np.uint32(0xFFFFF000)
    return u.view(np.float32)


def _wrap16(v: np.ndarray, L: int) -> np.ndarray:
    """[L] index list -> [128, L/16] int16 (16-lane wrap, replicated 8x)."""
    a = np.zeros(L, np.int16)
    a[: len(v)] = v
    return np.tile(a.reshape(L // 16, 16).T, (8, 1))


def _cols_wrap(v: np.ndarray, L: int) -> np.ndarray:
    """[cnt] col list -> [128, L/128] fp32 (slot m -> [m%128, m//128]), pad -1."""
    a = np.full(L, -1.0, np.float32)
    a[: len(v)] = v
    return a.reshape(L // 128, 128).T


def preprocess(x, edge_index, W1, b1, ln1_g, ln1_b, sc_W, sc_b, bn_g, bn_b,
               gcn_W, gcn_b, ln2_g, ln2_b, n_cores=8, split=32768):
    """Shard inputs; returns (in_maps, L_lo, L_hi, cfg)."""
    x = np.asarray(x, np.float32)
    ei = np.asarray(edge_index)
    N, D0 = x.shape
    D1 = W1.shape[0]
    D2 = gcn_W.shape[0]
    P = n_cores
    NC = N // P
    NPAD = ((NC + 127) // 128) * 128
    NB = NPAD // 128
    TBL = P * NPAD

    # node-quarter chunking of the AllGather: quarter q covers blocks
    # [qs[q], qs[q+1]); table rows are laid out quarter-major then core-major.
    nq = min(NQ, NB)
    # front-load quarters so the last AllGather (which gates the first
    # gather) moves the least data
    if nq == NQ and NB > 2 * nq:
        last = 2
        big = (NB - last) // (nq - 1)
        szs = [big] * (nq - 1) + [NB - last - big * (nq - 2)]
        szs[-1] = NB - sum(szs[:-1])
        qs = [0]
        for z in szs:
            qs.append(qs[-1] + z)
    else:
        qs = [round(q * NB / nq) for q in range(nq + 1)]
    qsz = np.array([qs[q + 1] - qs[q] for q in range(nq)])
    qbase = np.cumsum([0] + [P * int(s) * 128 for s in qsz])
    q_of_block = np.repeat(np.arange(nq), qsz)
    qs_arr = np.array(qs[:nq])

    # self-loops handled locally in pass 3 (identity matmul of the block's
    # own xws rows); only real edges go through the gather. Degree still
    # counts the self-loop (A+I normalization).
    loops = np.arange(N, dtype=np.int64)
    row = ei[0].astype(np.int64)
    col = ei[1].astype(np.int64)

    deg = np.bincount(np.concatenate([col, loops]),
                      minlength=N).astype(np.float64)
    dinv = (1.0 / np.sqrt(deg)).astype(np.float32)  # deg >= 1 (self-loops)

    sown = row // NC
    sloc = row - sown * NC
    sblk = sloc >> 7
    sq = q_of_block[sblk]
    trow = qbase[sq] + sown * (qsz[sq] * 128) + (sloc - qs_arr[sq] * 128)
    owner = col // NC
    lcol = col - owner * NC

    # bucket edges per (core, target block, lo/hi table half)
    per = []
    for c in range(P):
        m = owner == c
        tr, lc = trow[m], lcol[m]
        blk = lc >> 7
        hi = (tr >= split).astype(np.int64)
        order = np.lexsort((tr, hi, blk))
        tr, lc, blk, hi = tr[order], lc[order], blk[order], hi[order]
        bounds = np.searchsorted(blk * 2 + hi, np.arange(2 * NB + 1))
        blocks = []
        for b in range(NB):
            lo_s, lo_e = bounds[2 * b], bounds[2 * b + 1]
            hi_s, hi_e = bounds[2 * b + 1], bounds[2 * b + 2]
            blocks.append((tr[lo_s:lo_e], tr[hi_s:hi_e] - split,
                           lc[lo_s:lo_e] - b * 128, lc[hi_s:hi_e] - b * 128))
        per.append(blocks)

    def rup128(n):
        return ((n + 127) // 128) * 128

    L_lo = [rup128(max(len(per[c][b][0]) for c in range(P))) for b in range(NB)]
    L_hi = [rup128(max(len(per[c][b][1]) for c in range(P))) for b in range(NB)]

    TI = sum(L_lo[b] + L_hi[b] for b in range(NB)) // 16
    TG = sum(L_lo[b] + L_hi[b] for b in range(NB)) // 128
    Gmax = max((L_lo[b] + L_hi[b]) // 128 for b in range(NB))

    from ml_dtypes import bfloat16
    

    KC0 = D0 // 128
    w1t = np.ascontiguousarray(W1.T).astype(bfloat16)      # [D0, D1]
    scwt = np.ascontiguousarray(sc_W.T).astype(bfloat16)   # [D0, D1]
    gcnwt = np.ascontiguousarray(gcn_W.T).astype(bfloat16)  # [D1, D2]
    iota = np.broadcast_to(np.arange(128, dtype=np.float32), (128, 128))
    iota_big = np.ascontiguousarray(np.tile(iota, (1, Gmax))).astype(bfloat16)
    ident = np.eye(128, dtype=np.float32).astype(bfloat16)
    ones = np.ones((128, 1), np.float32)

    vec = lambda v: np.asarray(v, np.float32).reshape(1, -1)

    in_maps = []
    for c in range(P):
        xp = np.zeros((D0, NPAD), np.float32)
        xp[:, :NC] = x[c * NC:(c + 1) * NC].T
        # pretile: [NB, p(d0 in k-chunk), k, n] so each chunk is one
        # contiguous [128, KC*128] DMA
        xp = np.ascontiguousarray(
            xp.reshape(KC0, 128, NB, 128).transpose(2, 1, 0, 3)).astype(bfloat16)

        dv = np.zeros(NPAD, np.float32)
        dv[:NC] = dinv[c * NC:(c + 1) * NC]
        dvT = np.ascontiguousarray(dv.reshape(NB, 128).T)  # [128, NB]

        idx_all = np.zeros((128, TI), np.int16)
        cols_all = np.full((128, TG), -1.0, np.float32)
        counts = []

        def seg_idx(vals, L):
            a = np.full(L, -1, np.int16)
            a[: len(vals)] = vals
            o = 0
            while o < L:
                n = min(GATHER_SUB, L - o)
                cnt = min(max(len(vals) - o, 16), n)
                a[max(o, len(vals)): o + cnt] = 0   # padded-but-gathered
                counts.append(cnt)
                o += n
            return a

        oi = og = 0
        for b in range(NB):
            lo_t, hi_t, lo_c, hi_c = per[c][b]
            ll, lh = L_lo[b], L_hi[b]
            av = seg_idx(lo_t.astype(np.int16), ll)
            idx_all[:, oi:oi + ll // 16] = np.tile(av.reshape(ll // 16, 16).T, (8, 1))
            if lh:
                bv = seg_idx(hi_t.astype(np.int16), lh)
                idx_all[:, oi + ll // 16: oi + (ll + lh) // 16] = np.tile(
                    bv.reshape(lh // 16, 16).T, (8, 1))
            cols_all[:, og:og + ll // 128] = _cols_wrap(lo_c.astype(np.float32), ll)
            cols_all[:, og + ll // 128: og + (ll + lh) // 128] = _cols_wrap(hi_c.astype(np.float32), lh)
            oi += (ll + lh) // 16
            og += (ll + lh) // 128
        gcnt = np.asarray(counts, np.int32).reshape(1, -1)

        cols_all = cols_all.astype(bfloat16)

        in_maps.append({
            "xt": xp, "w1t": w1t, "scwt": scwt, "gcnwt": gcnwt,
            "b1": vec(b1), "ln1_g": vec(ln1_g), "ln1_b": vec(ln1_b),
            "bn_g": vec(bn_g), "bn_b": vec(bn_b),
            "gcn_b": vec(gcn_b), "ln2_g": vec(ln2_g), "ln2_b": vec(ln2_b),
            "idx": idx_all, "cols": cols_all, "dinvT": dvT, "gcnt": gcnt,
            "iota_big": iota_big, "ident": ident, "ones": ones,
        })

    cfg = dict(P=P, N=N, NC=NC, NPAD=NPAD, NB=NB, D0=D0, D1=D1, D2=D2,
               NCALLS=len(counts),
               TBL=TBL, SPLIT=split, TI=TI, TG=TG, Gmax=Gmax,
               qs=[int(v) for v in qs], qsz=[int(v) for v in qsz],
               qbase=[int(v) for v in qbase])
    return in_maps, L_lo, L_hi, cfg


def build_program(cfg, L_lo, L_hi, table_bf16=True):
    P, N = cfg["P"], cfg["N"]
    NPAD, NB = cfg["NPAD"], cfg["NB"]
    D0, D1, D2 = cfg["D0"], cfg["D1"], cfg["D2"]
    TBL, SPLIT, Gmax = cfg["TBL"], cfg["SPLIT"], cfg["Gmax"]
    qs, qsz, qbase = cfg["qs"], cfg["qsz"], cfg["qbase"]
    nq = len(qsz)
    KC, JC = D0 // 128, D1 // 128
    rg = [list(range(P))]
    TDT = BF16 if table_bf16 else F32R

    nc = bacc.Bacc("TRN2", target_bir_lowering=False, debug=False, num_devices=P,
                   num_swdge_queues=4)

    xt_d = nc.dram_tensor("xt", [NB, 128, KC, 128], BF16, kind="ExternalInput").ap()
    w1t_d = nc.dram_tensor("w1t", [D0, D1], BF16, kind="ExternalInput").ap()
    scwt_d = nc.dram_tensor("scwt", [D0, D1], BF16, kind="ExternalInput").ap()
    gcnwt_d = nc.dram_tensor("gcnwt", [D1, D2], BF16, kind="ExternalInput").ap()
    b1_d = nc.dram_tensor("b1", [1, D1], F32, kind="ExternalInput").ap()
    ln1g_d = nc.dram_tensor("ln1_g", [1, D1], F32, kind="ExternalInput").ap()
    ln1b_d = nc.dram_tensor("ln1_b", [1, D1], F32, kind="ExternalInput").ap()
    bng_d = nc.dram_tensor("bn_g", [1, D1], F32, kind="ExternalInput").ap()
    bnb_d = nc.dram_tensor("bn_b", [1, D1], F32, kind="ExternalInput").ap()
    gcnb_d = nc.dram_tensor("gcn_b", [1, D2], F32, kind="ExternalInput").ap()
    ln2g_d = nc.dram_tensor("ln2_g", [1, D2], F32, kind="ExternalInput").ap()
    ln2b_d = nc.dram_tensor("ln2_b", [1, D2], F32, kind="ExternalInput").ap()
    idx_d = nc.dram_tensor("idx", [128, cfg["TI"]], I16, kind="ExternalInput").ap()
    cols_d = nc.dram_tensor("cols", [128, cfg["TG"]], BF16, kind="ExternalInput").ap()
    dinv_d = nc.dram_tensor("dinvT", [128, NB], F32, kind="ExternalInput").ap()
    iotab_d = nc.dram_tensor("iota_big", [128, Gmax * 128], BF16, kind="ExternalInput").ap()
    ident_d = nc.dram_tensor("ident", [128, 128], BF16, kind="ExternalInput").ap()
    ones_d = nc.dram_tensor("ones", [128, 1], F32, kind="ExternalInput").ap()
    gcnt_d = nc.dram_tensor("gcnt", [1, cfg["NCALLS"]], mybir.dt.int32,
                            kind="ExternalInput").ap()
    out_d = nc.dram_tensor("out", [NPAD, D2], F32, kind="ExternalOutput").ap()

    xwsq = [nc.dram_tensor(f"xwsq{q}", [qsz[q] * 128, D2], TDT) for q in range(nq)]
    table = nc.dram_tensor("table", [TBL, D2], TDT, addr_space="Shared")
    bn_in = nc.dram_tensor("bn_in", [1, 2 * D1], F32)
    bn_out = nc.dram_tensor("bn_out", [1, 2 * D1], F32, addr_space="Shared")
    ab_d = nc.dram_tensor("ab_d", [1, 2 * D1], F32)      # A|Ball bounce

    r = lambda ap: ap.bitcast(F32R)

    with tile.TileContext(nc) as tc, ExitStack() as ctx:
        const = ctx.enter_context(tc.tile_pool(name="const", bufs=1))

        def const_load(name, dram_ap, shape, dt=F32, bcast=False):
            t = const.tile(shape, dt, tag=name)
            src = dram_ap.to_broadcast(shape) if bcast else dram_ap
            if dt == F32R:
                src = src.bitcast(F32R)
            nc.sync.dma_start(t[:], src)
            return t

        w1t_sb = const.tile([128, KC, D1], BF16, tag="w1t_sb")
        nc.sync.dma_start(w1t_sb[:], w1t_d.rearrange("(k p) n -> p k n", p=128))
        scwt_sb = const.tile([128, KC, D1], BF16, tag="scwt_sb")
        nc.sync.dma_start(scwt_sb[:], scwt_d.rearrange("(k p) n -> p k n", p=128))
        gcnwt_sb = const.tile([128, JC, D2], BF16, tag="gcnwt_sb")
        nc.sync.dma_start(gcnwt_sb[:], gcnwt_d.rearrange("(k p) n -> p k n", p=128))

        b1_t = const_load("b1_t", b1_d, [128, D1], bcast=True)
        ln1g_t = const_load("ln1g_t", ln1g_d, [128, D1], bcast=True)
        gcnb_t = const_load("gcnb_t", gcnb_d, [128, D2], bcast=True)
        ln2g_t = const_load("ln2g_t", ln2g_d, [128, D2], bcast=True)
        ln2b_t = const_load("ln2b_t", ln2b_d, [128, D2], bcast=True)
        # pass-3-only constants go on the scalar DMA queue so the sync queue
        # reaches pass 1's xt loads sooner
        iotab_c = const.tile([128, Gmax * 128], BF16, tag="iotab_c")
        nc.scalar.dma_start(iotab_c[:], iotab_d)
        ident_sb = const_load("ident_sb", ident_d, [128, 128], dt=BF16)
        ones_sb = const_load("ones_sb", ones_d, [128, 1])
        dinv_sb = const_load("dinv_sb", dinv_d, [128, NB])
        ln1b_row = const_load("ln1b_row", ln1b_d, [1, D1])
        bng_row = const_load("bng_row", bng_d, [1, D1])
        bnb_row = const_load("bnb_row", bnb_d, [1, D1])
        idx_all_sb = const.tile([128, cfg["TI"]], I16, tag="idx_all_sb")
        nc.scalar.dma_start(idx_all_sb[:], idx_d[:])
        gcnt_sb = const.tile([1, cfg["NCALLS"]], mybir.dt.int32, tag="gcnt_sb")
        nc.sync.dma_start(gcnt_sb[:], gcnt_d[:])
        cols_all_sb = const.tile([128, cfg["TG"]], BF16, tag="cols_all_sb")
        nc.scalar.dma_start(cols_all_sb[:], cols_d[:])

        eps_sb = const.tile([128, 1], F32, tag="eps_sb")
        nc.vector.memset(eps_sb[:], LN_EPS)
        eps1_sb = const.tile([1, 1], F32, tag="eps1_sb")
        nc.vector.memset(eps1_sb[:], BN_EPS)

        abt_t = const.tile([128, 2 * D1], F32, tag="abt_t")
        A_t = abt_t[:, 0:D1]
        Ball_t = abt_t[:, D1:2 * D1]

        xt_v = xt_d  # [NB, 128, KC, 128] chunk-contiguous, bf16

        p12 = ExitStack()
        resid = p12.enter_context(tc.tile_pool(name="resid", bufs=1))
        fh_r = resid.tile([128, NB, D1], BF16, tag="fh_r", name="fh_r")
        ysc_r = resid.tile([128, NB, D1], BF16, tag="ysc_r", name="ysc_r")

        # ---- pass 1a: ysc = x @ scW.T ; BN sums via vector accumulation --
        # (the tiny BN AllReduce is kicked right after, so it overlaps the
        # y1 GEMM of pass 1b instead of stalling the whole device)
        bn_sb = const.tile([1, 2 * D1], F32, tag="bn_sb")
        S_acc = const.tile([128, D1], F32, tag="S_acc")
        Q_acc = const.tile([128, D1], F32, tag="Q_acc")
        nc.vector.memset(S_acc[:], 0.0)
        nc.vector.memset(Q_acc[:], 0.0)
        with ExitStack() as p1:
            xpool = p1.enter_context(tc.tile_pool(name="xpool", bufs=3))
            work = p1.enter_context(tc.tile_pool(name="work1", bufs=3))
            ps = p1.enter_context(tc.tile_pool(name="ps1", bufs=2, space="PSUM"))

            for i in range(NB):
                xt = xpool.tile([128, KC, 128], BF16, tag="xt")
                nc.sync.dma_start(xt[:], xt_v[i])
                yscp = ps.tile([128, D1], F32, tag="yscp")
                for k in range(KC):
                    nc.tensor.matmul(yscp[:], xt[:, k, :], scwt_sb[:, k, :],
                                     start=(k == 0), stop=(k == KC - 1))
                ysc = ysc_r[:, i, :]
                nc.scalar.copy(ysc, yscp[:])
                sq = work.tile([128, D1], BF16, tag="sq")
                nc.gpsimd.tensor_mul(sq[:], ysc, ysc)
                nc.vector.tensor_add(S_acc[:], S_acc[:], yscp[:])
                nc.vector.tensor_add(Q_acc[:], Q_acc[:], sq[:])

            sum_ps = p1.enter_context(tc.tile_pool(name="sum_ps", bufs=1, space="PSUM"))
            sump = sum_ps.tile([1, D1], F32, tag="sump")
            sqsump = sum_ps.tile([1, D1], F32, tag="sqsump")
            nc.tensor.matmul(sump[:], ones_sb[:], S_acc[:], start=True, stop=True)
            nc.tensor.matmul(sqsump[:], ones_sb[:], Q_acc[:], start=True, stop=True)
            nc.vector.tensor_copy(bn_sb[:, 0:D1], sump[:])
            nc.vector.tensor_copy(bn_sb[:, D1:2 * D1], sqsump[:])

            nc.scalar.dma_start(bn_in.ap()[:], bn_sb[:])
            nc.gpsimd.collective_compute(
                "AllReduce", mybir.AluOpType.add, replica_groups=rg,
                ins=[bn_in.ap()[:]], outs=[bn_out.ap()[:]])

            # ---- pass 1b: y1 = x @ W1.T ; LayerNorm1 (overlaps the AR;
            # same pools as 1a so the xt prefetch crosses the boundary)
            for i in range(NB):
                xt = xpool.tile([128, KC, 128], BF16, tag="xt")
                nc.sync.dma_start(xt[:], xt_v[i])
                y1p = ps.tile([128, D1], F32, tag="y1p")
                for k in range(KC):
                    nc.tensor.matmul(y1p[:], xt[:, k, :], w1t_sb[:, k, :],
                                     start=(k == 0), stop=(k == KC - 1))
                # LayerNorm1 (per-node, no cross-core stats needed)
                y1b = work.tile([128, D1], F32, tag="y1b")
                nc.vector.tensor_add(y1b[:], y1p[:], b1_t[:])
                st = work.tile([128, 6], F32, tag="st")
                nc.vector.bn_stats(st[:], y1b[:])
                mv = work.tile([128, 2], F32, tag="mv")
                nc.vector.bn_aggr(mv[:], st[:])
                nc.scalar.activation(mv[:, 1:2], mv[:, 1:2],
                                     mybir.ActivationFunctionType.Sqrt,
                                     bias=eps_sb[:])
                nc.vector.reciprocal(mv[:, 1:2], mv[:, 1:2])
                nmr = work.tile([128, 1], F32, tag="nmr")
                nc.vector.tensor_scalar(nmr[:], mv[:, 0:1], mv[:, 1:2], -1.0,
                                        op0=mybir.AluOpType.mult,
                                        op1=mybir.AluOpType.mult)
                f = work.tile([128, D1], F32, tag="f")
                nc.scalar.activation(f[:], y1b[:],
                                     mybir.ActivationFunctionType.Identity,
                                     bias=nmr[:], scale=mv[:, 1:2])
                nc.gpsimd.tensor_mul(fh_r[:, i, :], f[:], ln1g_t[:])

        # ---- BatchNorm stats: read AR result, form A/Ball vectors --------
        bnall = const.tile([1, 2 * D1], F32, tag="bnall")
        nc.scalar.dma_start(bnall[:], bn_out.ap()[:])

        mean_r = const.tile([1, D1], F32, tag="mean_r")
        nc.scalar.mul(mean_r[:], bnall[:, 0:D1], 1.0 / N)
        var_r = const.tile([1, D1], F32, tag="var_r")
        nc.scalar.mul(var_r[:], bnall[:, D1:2 * D1], 1.0 / N)
        msq = const.tile([1, D1], F32, tag="msq")
        nc.vector.tensor_mul(msq[:], mean_r[:], mean_r[:])
        nc.vector.tensor_sub(var_r[:], var_r[:], msq[:])
        nc.scalar.activation(var_r[:], var_r[:],
                             mybir.ActivationFunctionType.Sqrt, bias=eps1_sb[:])
        nc.vector.reciprocal(var_r[:], var_r[:])          # rstd
        ab_row = const.tile([1, 2 * D1], F32, tag="ab_row")
        A_row = ab_row[:, 0:D1]
        ball_row = ab_row[:, D1:2 * D1]
        nc.vector.tensor_mul(A_row, var_r[:], bng_row[:])
        mA = const.tile([1, D1], F32, tag="mA")
        nc.vector.tensor_mul(mA[:], mean_r[:], A_row)
        nc.vector.tensor_sub(ball_row, bnb_row[:], mA[:])
        nc.vector.tensor_add(ball_row, ball_row, ln1b_row[:])
        nc.scalar.dma_start(ab_d.ap()[0:1, :], ab_row[:])
        nc.scalar.dma_start(abt_t[:], ab_d.ap()[0:1, :].to_broadcast([128, 2 * D1]))
        abt16 = const.tile([128, 2 * D1], BF16, tag="abt16")
        nc.vector.tensor_copy(abt16[:], abt_t[:])
        A16 = abt16[:, 0:D1]
        Ball16 = abt16[:, D1:2 * D1]

        # ---- pass 2: y1, LN1, h, hT, xw, scale, store (+chunked AG) ------
        with ExitStack() as p2:
            xpool = p2.enter_context(tc.tile_pool(name="xpool2", bufs=3))
            work = p2.enter_context(tc.tile_pool(name="work2", bufs=6))
            ps = p2.enter_context(tc.tile_pool(name="ps2", bufs=2, space="PSUM"))
            tps = p2.enter_context(tc.tile_pool(name="tps", bufs=3, space="PSUM"))

            q_cur = 0
            for i in range(NB):
                t = work.tile([128, D1], BF16, tag="t")
                nc.gpsimd.tensor_mul(t[:], ysc_r[:, i, :], A16)
                f = work.tile([128, D1], BF16, tag="f")
                nc.vector.tensor_add(f[:], t[:], fh_r[:, i, :])
                nc.vector.tensor_add(f[:], f[:], Ball16)
                # Lrelu is safe here: pass 2's only table-based activation
                h = work.tile([128, D1], BF16, tag="h")
                nc.scalar.activation(h[:], f[:],
                                     mybir.ActivationFunctionType.Lrelu,
                                     alpha=SLOPE)

                ht = work.tile([128, JC, 128], BF16, tag="ht")
                for j in range(JC):
                    tp = tps.tile([128, 128], BF16, tag="tp")
                    nc.tensor.transpose(tp[:], h[:, j * 128:(j + 1) * 128], ident_sb[:])
                    if j % 2 == 0:
                        nc.scalar.copy(ht[:, j, :], tp[:])
                    else:
                        nc.vector.tensor_copy(ht[:, j, :], tp[:])
                xwp = ps.tile([128, D2], F32, tag="xwp")
                for j in range(JC):
                    nc.tensor.matmul(xwp[:], ht[:, j, :], gcnwt_sb[:, j, :],
                                     start=(j == 0), stop=(j == JC - 1))
                xws = work.tile([128, D2], TDT, tag="xws")
                nc.vector.tensor_scalar(xws[:], xwp[:], dinv_sb[:, i:i + 1], None,
                                        op0=mybir.AluOpType.mult)
                q = q_cur
                nc.sync.dma_start(
                    xwsq[q].ap()[(i - qs[q]) * 128:(i - qs[q] + 1) * 128, :],
                    xws[:])
                if i + 1 == qs[q_cur + 1]:
                    # quarter complete: AllGather it into its table slice
                    nc.gpsimd.collective_compute(
                        "AllGather", mybir.AluOpType.bypass, replica_groups=rg,
                        ins=[xwsq[q].ap()[:]],
                        outs=[table.ap()[qbase[q]:qbase[q + 1], :]])
                    q_cur += 1

        p12.close()

        # ---- pass 3: gather + S-matmul aggregation + LN2 -----------------
        with ExitStack() as p3:
            nb3 = 3 if table_bf16 else 2
            mpool = p3.enter_context(tc.tile_pool(name="mpool", bufs=1))
            spool = p3.enter_context(tc.tile_pool(name="spool", bufs=nb3))
            work = p3.enter_context(tc.tile_pool(name="work3", bufs=3))
            ps = p3.enter_context(tc.tile_pool(name="ps3", bufs=4, space="PSUM"))

            # persistent msg ring, zeroed once: dynamic-count gathers leave
            # tail slots unwritten, and PSUM matmuls must never see NaN
            msg_ring = [mpool.tile([128, Gmax, D2], TDT, tag=f"msgr{r}", name=f"msgr{r}")
                        for r in range(6)]
            for mt in msg_ring:
                nc.vector.memset(mt[:], 0.0)

            # bulk-reload this core's scaled rows (self-loop term) from the
            # quarter scratch; one DMA per quarter, off the per-block path
            xws_r = mpool.tile([128, NB, D2], TDT, tag="xws_r", name="xws_r")
            for q in range(len(qsz)):
                nc.scalar.dma_start(
                    xws_r[:, qs[q]:qs[q + 1], :],
                    xwsq[q].ap().rearrange("(b p) d -> p b d", p=128))

            qc = 0  # SWDGE queue round-robin
            oi = og = 0
            for b in range(NB):
                ll, lh = L_lo[b], L_hi[b]
                G = (ll + lh) // 128
                idx_sb = idx_all_sb[:, oi:oi + (ll + lh) // 16]
                cols_sb = cols_all_sb[:, og:og + G]

                msg = msg_ring[b % 6][:, 0:G, :]
                for base, ln in ((0, ll), (ll, lh)):
                    if ln == 0:
                        continue
                    tbl = table.ap()[0:SPLIT, :] if base == 0 else table.ap()[SPLIT:TBL, :]
                    o = 0
                    while o < ln:
                        n = min(GATHER_SUB, ln - o)
                        with nc.gpsimd.register() as creg:
                            nc.gpsimd.load(creg, gcnt_sb[0:1, qc:qc + 1])
                            nc.gpsimd.dma_gather(
                                out_ap=msg[:, (base + o) // 128:(base + o + n) // 128, :],
                                in_ap=tbl,
                                idxs_ap=idx_sb[:, (base + o) // 16:(base + o + n) // 16],
                                num_idxs=n, num_idxs_reg=creg, elem_size=D2,
                                single_packet=True, queue_num=qc % 4)
                        qc += 1
                        o += n

                S_all = spool.tile([128, G, 128], TDT, tag="S_all")
                nc.vector.tensor_tensor(
                    out=S_all[:],
                    in0=iotab_c[:, 0:G * 128].rearrange("p (g t) -> p g t", g=G),
                    in1=cols_sb[:, 0:G].unsqueeze(2).broadcast_to((128, G, 128)),
                    op=mybir.AluOpType.is_equal)

                acc = ps.tile([128, D2], F32, tag="acc")
                for g in range(G):
                    nc.tensor.matmul(acc[:], S_all[:, g, :], msg[:, g, :],
                                     start=(g == 0), stop=False)
                # self-loop: acc[t,:] += xws[t,:] via identity matmul
                nc.tensor.matmul(acc[:], ident_sb[:], xws_r[:, b, :],
                                 start=False, stop=True)

                ev = work.tile([128, D2], F32, tag="ev")
                nc.vector.tensor_scalar(ev[:], acc[:], dinv_sb[:, b:b + 1], None,
                                        op0=mybir.AluOpType.mult)
                nc.vector.tensor_add(ev[:], ev[:], gcnb_t[:])
                st = work.tile([128, 6], F32, tag="st3")
                nc.vector.bn_stats(st[:], ev[:])
                mv = work.tile([128, 2], F32, tag="mv3")
                nc.vector.bn_aggr(mv[:], st[:])
                nc.scalar.activation(mv[:, 1:2], mv[:, 1:2],
                                     mybir.ActivationFunctionType.Sqrt,
                                     bias=eps_sb[:])
                nc.vector.reciprocal(mv[:, 1:2], mv[:, 1:2])
                nmr = work.tile([128, 1], F32, tag="nmr3")
                nc.vector.tensor_scalar(nmr[:], mv[:, 0:1], mv[:, 1:2], -1.0,
                                        op0=mybir.AluOpType.mult,
                                        op1=mybir.AluOpType.mult)
                f2 = work.tile([128, D2], F32, tag="f2")
                nc.scalar.activation(f2[:], ev[:],
                                     mybir.ActivationFunctionType.Identity,
                                     bias=nmr[:], scale=mv[:, 1:2])
                nc.vector.tensor_mul(f2[:], f2[:], ln2g_t[:])
                nc.vector.tensor_add(f2[:], f2[:], ln2b_t[:])
                ol = work.tile([128, D2], F32, tag="ol")
                nc.scalar.mul(ol[:], f2[:], SLOPE)
                oo = work.tile([128, D2], F32, tag="oo")
                nc.vector.tensor_max(oo[:], f2[:], ol[:])
                nc.sync.dma_start(out_d[b * 128:(b + 1) * 128, :], oo[:])

                oi += (ll + lh) // 16
                og += G

    nc.compile()
    return nc


_last_results = None


def kernel(**inputs) -> np.ndarray:
    global _last_results
    in_maps, L_lo, L_hi, cfg = preprocess(**inputs)
    table_bf16 = os.environ.get("GNN_TABLE_FP32", "") != "1"
    nc = build_program(cfg, L_lo, L_hi, table_bf16=table_bf16)
    trace = os.environ.get("GNN_TRACE", "") == "1"
    res = run_bass_kernel_spmd(nc, in_maps, core_ids=list(range(cfg["P"])),
                               trace=trace)
    _last_results = res
    NC = cfg["NC"]
    return np.concatenate([res.results[c]["out"][:NC] for c in range(cfg["P"])],
                          axis=0)

